# revision 1
# baseline (speedup 1.0000x reference)
"""MDTA (channel-attention transformer block) Trainium2 kernel, v3.

Math (zero-bias fast path; x16 = fp16(x), per-token mu/r from x16):
  G_needed = sum_t r^2 (x-mu*1)(x-mu*1)^T = G2 - u 1^T - 1 u^T + s 1 1^T
    G2 = sum r^2 x x^T = (r^2 x)^T X,  u = sum r^2 mu x,  s = 1^T u / C
  scores = wk'^T G wq'/alpha (diag 32x32 blocks), attn = softmax
  W2 = diag(g) Wv blockdiag(attn) Wf + diag(gamma),  w2s = 1^T W2
  y_t = r_t(W2^T x16_t) - r_t mu_t w2s = W2^T(x*rb) - w2s (x) rmu_row

Structure (engine-measured costs drove every choice):
  Phase A streams xT16 ([c, t], host-transposed): DVE squares each chunk;
  PE accumulates per-token sums sum(x), sum(x^2) as ROWS of two [64, 512]
  PSUM tiles using a shifted ones-column selector matrix as lhsT (row q =
  chunk q).  Stats math then runs batched on [64, 512] (a few DVE/ACT ops),
  giving r/r^2/mu/rmu in both row form (phase 3) and, via 8 PE transposes,
  column form (phase B).
  Phase B streams x_nat ([t, c]): one broadcast multiply zr2 = x * r^2 per
  group; Gram matmuls G2 += zr2^T [x | mu] with the mu column appended in
  SBUF by ACT.  [G2 | u] is all-reduced pairwise; the rank-2 mean correction
  is applied in (packed) score space; softmax -> W2 fp16, w2s.
  Phase 3 re-streams xT16: xts = xt * (1 (x) r_row) (DVE, PE builds the
  broadcast); yp = W2^T xts - w2s (x) rmu_row in PSUM; a casting gpsimd DMA
  writes yp straight to fp16 DRAM.

Sharding: 8 cores = (batch 0..3) x (token half 0..1); 66 KB pair all-reduce.
Host does layout/dtype staging only (fp16 casts, the [c, t] transpose,
gamma/alpha folding, final yT.T -> fp32).
"""

import sys

import numpy as np

for _p in ("/opt/trn_rl_repo",):
    if _p not in sys.path:
        sys.path.append(_p)

import concourse.bacc as bacc
import concourse.bass as bass
import concourse.tile as tile
from concourse import mybir
from concourse.bass_utils import run_bass_kernel_spmd

B, HH, WW, C = 4, 256, 256, 128
NH, S = 4, 32
T = HH * WW
N_CORES = 8
TLOC = T // 2
EPS = 1e-5
P = 128
GRP = 4
YC = 512

F32 = mybir.dt.float32
F16 = mybir.dt.float16

AF = mybir.ActivationFunctionType
OP = mybir.AluOpType
AX = mybir.AxisListType


def _bc(ap, n):
    """Append an inner stride-0 (broadcast) dim of size n to an AP."""
    return bass.AP(ap.tensor, ap.offset, list(ap.ap) + [[0, n]])


def build_nc(tloc=TLOC, n_cores=N_CORES):
    assert tloc % (P * GRP) == 0 and tloc % YC == 0
    nc = bacc.Bacc("TRN2", target_bir_lowering=False, debug=False,
                   num_devices=n_cores)

    ngrp = tloc // (P * GRP)
    nyc = tloc // YC          # chunks; also the sum-row count (<= 64)
    assert nyc <= 64

    x_in = nc.declare_dram_parameter("x_nat", [ngrp, P, GRP * C], F16,
                                     isOutput=False)
    xt_in = nc.declare_dram_parameter("x_tr", [C, tloc], F16, isOutput=False)
    wq_in = nc.declare_dram_parameter("wq_g", [C, C], F32, isOutput=False)
    wk_in = nc.declare_dram_parameter("wk_g", [C, C], F32, isOutput=False)
    wvT_in = nc.declare_dram_parameter("wvT4", [S, NH * C], F32, isOutput=False)
    wf_in = nc.declare_dram_parameter("wf", [C, C], F32, isOutput=False)
    dg_in = nc.declare_dram_parameter("diag_gamma", [C, C], F32, isOutput=False)
    id32_in = nc.declare_dram_parameter("ident_f32", [P, P], F32, isOutput=False)
    id16_in = nc.declare_dram_parameter("ident_f16", [P, P], F16, isOutput=False)
    w1q_in = nc.declare_dram_parameter("w1q_pk", [C, S], F32, isOutput=False)
    k1_in = nc.declare_dram_parameter("k1_col", [C, 2], F32, isOutput=False)
    hsel_in = nc.declare_dram_parameter("hsel", [NH, C], F32, isOutput=False)
    eq_in = nc.declare_dram_parameter("eqsel", [P, 2 * nyc - 1], F16,
                                      isOutput=False)
    on16_in = nc.declare_dram_parameter("ones16", [P, P], F16, isOutput=False)
    on32_in = nc.declare_dram_parameter("ones32", [P, P], F32, isOutput=False)
    yT_out = nc.declare_dram_parameter("yT16", [C, tloc], F16, isOutput=True)

    x_tiles = x_in.rearrange("g p (j c) -> g p j c", j=GRP)
    replica_groups = [[2 * b, 2 * b + 1] for b in range(n_cores // 2)]

    with tile.TileContext(nc) as tc:
        with (
            tc.tile_pool(name="const", bufs=1) as const,
            tc.tile_pool(name="xtload", bufs=4) as xtload,
            tc.tile_pool(name="sqbuf", bufs=4) as sqbuf,
            tc.tile_pool(name="xload", bufs=4) as xload,
            tc.tile_pool(name="small", bufs=2) as small,
            tc.tile_pool(name="dram", bufs=1, space="DRAM") as dram,
        ):
            # ---- constants ----
            wq_sb = const.tile([C, C], F32)
            wk_sb = const.tile([C, C], F32)
            wvT_sb = const.tile([S, NH, C], F32)
            wf_sb = const.tile([C, C], F32)
            dg_sb = const.tile([C, C], F32)
            id32_sb = const.tile([P, P], F32)
            id16_sb = const.tile([P, P], F16)
            w1q_sb = const.tile([C, S], F32)
            k1_sb = const.tile([C, 2], F32)
            hsel_sb = const.tile([NH, C], F32)
            eq_sb = const.tile([P, 2 * nyc - 1], F16)
            on16_sb = const.tile([P, P], F16)
            on32_sb = const.tile([P, P], F32)
            nc.sync.dma_start(out=wq_sb, in_=wq_in[:])
            nc.sync.dma_start(out=wk_sb, in_=wk_in[:])
            nc.sync.dma_start(out=wvT_sb,
                              in_=wvT_in[:].rearrange("s (h c) -> s h c", h=NH))
            nc.sync.dma_start(out=wf_sb, in_=wf_in[:])
            nc.sync.dma_start(out=dg_sb, in_=dg_in[:])
            nc.sync.dma_start(out=id32_sb, in_=id32_in[:])
            nc.sync.dma_start(out=id16_sb, in_=id16_in[:])
            nc.sync.dma_start(out=w1q_sb, in_=w1q_in[:])
            nc.sync.dma_start(out=k1_sb, in_=k1_in[:])
            nc.sync.dma_start(out=hsel_sb, in_=hsel_in[:])
            nc.sync.dma_start(out=eq_sb, in_=eq_in[:])
            nc.sync.dma_start(out=on16_sb, in_=on16_in[:])
            nc.sync.dma_start(out=on32_sb, in_=on32_in[:])
            eps_sb = const.tile([P, 1], F32)
            nc.vector.memset(eps_sb, EPS)

            # stats row arrays [nyc, YC] (token t = 512*q + t')
            sx_sb = const.tile([nyc, YC], F32)
            sq_sb = const.tile([nyc, YC], F32)
            t1_sb = const.tile([nyc, YC], F32)
            v_sb = const.tile([nyc, YC], F32)
            std_sb = const.tile([nyc, YC], F32)
            rstd_sb = const.tile([nyc, YC], F32)
            mu16_sb = const.tile([nyc, YC], F16)
            rmu16_sb = const.tile([nyc, YC], F16)
            r16_sb = const.tile([nyc, YC], F16)
            r216_sb = const.tile([nyc, YC], F16)
            # column-layout stats for phase B: [:, j, gq] = tile (4*gq + j)
            ncolw = tloc // (GRP * P)     # = ngrp
            r2col = const.tile([P, GRP, ncolw], F16)
            mucol = const.tile([P, GRP, ncolw], F16)
            # full rows on partition 0 for phase 3
            rmu_row1 = const.tile([1, tloc], F16)
            r_row1 = const.tile([1, tloc], F16)

            ZRING = 8
            zr2 = const.tile([P, ZRING, GRP, C], F16)

            # ============ Phase A: per-token sums via PE ============
            npa = nyc // 2
            xt_a = [xtload.tile([C, 2, YC], F16, name=f"xta{i}", tag="xt")
                    for i in range(npa)]
            with tc.tile_pool(name="psS", bufs=1, space="PSUM") as psS:
                sx_ps = psS.tile([nyc, YC], F32, tag="sx")
                sq_ps = psS.tile([nyc, YC], F32, tag="sq")
                for i in range(npa):
                    nc.gpsimd.dma_start(
                        out=xt_a[i], in_=xt_in[:, 2 * i * YC:(2 * i + 2) * YC])
                    for k in range(2):
                        q = 2 * i + k
                        xtq = xt_a[i][:, k]
                        sqg = sqbuf.tile([C, YC], F16, name="sqg", tag="sq")
                        if q % 2 == 0:
                            nc.vector.tensor_tensor(out=sqg, in0=xtq, in1=xtq,
                                                    op=OP.mult)
                        else:
                            nc.scalar.square(out=sqg, in_=xtq)
                        eq_v = eq_sb[:, nyc - 1 - q:2 * nyc - 1 - q]
                        nc.tensor.matmul(sx_ps, lhsT=eq_v, rhs=xtq,
                                         start=(q == 0), stop=(q == nyc - 1))
                        nc.tensor.matmul(sq_ps, lhsT=eq_v, rhs=sqg,
                                         start=(q == 0), stop=(q == nyc - 1))
                nc.vector.tensor_copy(out=sx_sb, in_=sx_ps)
                nc.vector.tensor_copy(out=sq_sb, in_=sq_ps)

            # ---- batched stats math on [nyc, YC] ----
            # var*C = sq - sx^2/C; rstd = 1/sqrt(var+eps)
            nc.vector.tensor_tensor(out=t1_sb, in0=sx_sb, in1=sx_sb, op=OP.mult)
            nc.vector.scalar_tensor_tensor(out=v_sb, in0=t1_sb,
                                           scalar=float(-1.0 / C), in1=sq_sb,
                                           op0=OP.mult, op1=OP.add)
            nc.scalar.activation(out=std_sb, in_=v_sb, func=AF.Sqrt,
                                 bias=eps_sb[0:nyc, :], scale=float(1.0 / C))
            nc.vector.reciprocal(out=rstd_sb, in_=std_sb)
            nc.scalar.mul(out=mu16_sb, in_=sx_sb, mul=float(1.0 / C))
            nc.vector.tensor_tensor(out=rmu16_sb, in0=mu16_sb, in1=rstd_sb,
                                    op=OP.mult)
            nc.scalar.copy(out=r16_sb, in_=rstd_sb)
            nc.vector.tensor_tensor(out=r216_sb, in0=rstd_sb, in1=rstd_sb,
                                    op=OP.mult)

            # rows for phase 3 (stream-order remap, 1 DMA each)
            nc.sync.dma_start(out=rmu_row1, in_=rmu16_sb)
            nc.sync.dma_start(out=r_row1, in_=r16_sb)

            with tc.tile_pool(name="ps2", bufs=1, space="PSUM") as ps2:
                # column layout for phase B: transpose [nyc, 128]-slices
                id_h = id16_sb[0:nyc, 0:nyc]
                for j in range(GRP):
                    tpj = ps2.tile([P, nyc], F16, tag="tp")
                    nc.tensor.transpose(tpj, r216_sb[:, j * P:(j + 1) * P], id_h)
                    nc.scalar.copy(out=r2col[:, j, :], in_=tpj)
                    tpm = ps2.tile([P, nyc], F16, tag="tp")
                    nc.tensor.transpose(tpm, mu16_sb[:, j * P:(j + 1) * P], id_h)
                    nc.scalar.copy(out=mucol[:, j, :], in_=tpm)

                with tc.tile_pool(name="psG", bufs=1, space="PSUM") as psG:
                    G_ps = psG.tile([C, C], F32, tag="g")
                    u_ps = psG.tile([C, 1], F32, tag="u")
                    # ============ Phase B: Gram G2 = (r^2 x)^T [x | mu] ============
                    ngp = ngrp // 2
                    nlast = ngrp * GRP - 1
                    for i2 in range(ngp):
                        xg2 = xload.tile([P, 2, GRP, C], F16, name="xg2", tag="xg")
                        nc.sync.dma_start(
                            out=xg2,
                            in_=x_in[2 * i2:2 * i2 + 2].rearrange(
                                "g p (j c) -> p g j c", j=GRP))
                        for k in range(2):
                            g = 2 * i2 + k
                            r = g % ZRING
                            xg9 = xg2[:, k]
                            nc.vector.tensor_tensor(out=zr2[:, r], in0=xg9,
                                                    in1=_bc(r2col[:, :, g], C),
                                                    op=OP.mult)
                            for j in range(GRP):
                                i = g * GRP + j
                                nc.tensor.matmul(G_ps, lhsT=zr2[:, r, j],
                                                 rhs=xg9[:, j],
                                                 start=(i == 0), stop=(i == nlast))
                                nc.tensor.matmul(u_ps, lhsT=zr2[:, r, j],
                                                 rhs=mucol[:, j, g:g + 1],
                                                 start=(i == 0), stop=(i == nlast))

                    g_sb = small.tile([C, C + 1], F32)
                    nc.vector.tensor_copy(out=g_sb[:, 0:C], in_=G_ps)
                    nc.vector.tensor_copy(out=g_sb[:, C:C + 1], in_=u_ps)
                # ============ all-reduce [G2 | u] ============
                g_in_d = dram.tile([C, C + 1], F32)
                g_out_d = dram.tile([C, C + 1], F32)
                nc.gpsimd.dma_start(out=g_in_d, in_=g_sb)
                nc.gpsimd.collective_compute(
                    "AllReduce", OP.add, replica_groups=replica_groups,
                    ins=[g_in_d[:].opt()], outs=[g_out_d[:].opt()])

                # -- overlap: prefetch first phase-3 xT chunk-pairs (sync q) --
                xt_tiles = [xtload.tile([C, 2, YC], F16, name=f"xt{i}", tag="x3")
                            for i in range(npa)]
                npre = min(4, npa)
                for i in range(npre):
                    nc.sync.dma_start(out=xt_tiles[i],
                                      in_=xt_in[:, 2 * i * YC:(2 * i + 2) * YC])

                gs_sb = small.tile([C, C + 1], F32)
                nc.gpsimd.dma_start(out=gs_sb, in_=g_out_d)

                # ============ Phase 2: scores + softmax + W2 ============
                u_ap = gs_sb[:, C:C + 1]
                s1_ps = ps2.tile([C, C], F32, tag="mm")
                nc.tensor.matmul(s1_ps, lhsT=gs_sb[:, 0:C], rhs=wq_sb,
                                 start=True, stop=True)   # G symmetric (to fp16)
                s1_sb = small.tile([C, C], F32)
                nc.scalar.copy(out=s1_sb, in_=s1_ps)
                sc_ps = ps2.tile([C, C], F32, tag="mm")
                nc.tensor.matmul(sc_ps, lhsT=wk_sb, rhs=s1_sb, start=True, stop=True)
                spk = small.tile([P, S], F32)
                for h in range(NH):
                    nc.scalar.copy(out=spk[h * S:(h + 1) * S, :],
                                   in_=sc_ps[h * S:(h + 1) * S, h * S:(h + 1) * S])

                a_ps = ps2.tile([C, 1], F32, tag="sm")
                nc.tensor.matmul(a_ps, lhsT=wk_sb, rhs=u_ap, start=True, stop=True)
                a_sb = small.tile([C, 1], F32)
                nc.vector.tensor_copy(out=a_sb, in_=a_ps)
                bc_ps = ps2.tile([C, 1], F32, tag="sm")
                nc.tensor.matmul(bc_ps, lhsT=wq_sb, rhs=u_ap, start=True, stop=True)
                bc_sb = small.tile([C, 1], F32)
                nc.scalar.copy(out=bc_sb, in_=bc_ps)
                su_ps = ps2.tile([1, 1], F32, tag="sm")
                nc.tensor.matmul(su_ps, lhsT=u_ap, rhs=on32_sb[:, 0:1],
                                 start=True, stop=True)
                su_sb = small.tile([1, 1], F32)
                nc.scalar.copy(out=su_sb, in_=su_ps)
                sc_col_ps = ps2.tile([C, 1], F32, tag="sm")
                nc.tensor.matmul(sc_col_ps, lhsT=on32_sb[0:1, :], rhs=su_sb,
                                 start=True, stop=True)
                scol_sb = small.tile([C, 1], F32)
                nc.scalar.mul(out=scol_sb, in_=sc_col_ps, mul=float(1.0 / C))

                bT_ps = ps2.tile([1, C], F32, tag="sm")
                nc.tensor.transpose(bT_ps, bc_sb, id32_sb)
                bT_sb = small.tile([1, C], F32)
                nc.scalar.copy(out=bT_sb, in_=bT_ps)
                bT4_sb = small.tile([NH, S], F32)
                nc.sync.dma_start(out=bT4_sb, in_=bT_sb)
                bpk_ps = ps2.tile([C, S], F32, tag="sm")
                nc.tensor.matmul(bpk_ps, lhsT=hsel_sb, rhs=bT4_sb,
                                 start=True, stop=True)

                tmp_sb = small.tile([C, 1], F32)
                nc.vector.scalar_tensor_tensor(
                    out=tmp_sb, in0=scol_sb, scalar=k1_sb[:, 0:1], in1=a_sb,
                    op0=OP.mult, op1=OP.subtract)             # s*k1 - a
                s1c = small.tile([P, S], F32)
                nc.vector.scalar_tensor_tensor(
                    out=s1c, in0=w1q_sb, scalar=tmp_sb, in1=spk,
                    op0=OP.mult, op1=OP.add)
                scor = small.tile([P, S], F32)
                nc.vector.scalar_tensor_tensor(
                    out=scor, in0=bpk_ps, scalar=k1_sb[:, 1:2], in1=s1c,
                    op0=OP.mult, op1=OP.add)

                mx = small.tile([P, 1], F32)
                nc.vector.reduce_max(mx, scor, AX.X)
                nmx = small.tile([P, 1], F32)
                nc.vector.tensor_scalar_mul(out=nmx, in0=mx, scalar1=-1.0)
                sh = small.tile([P, S], F32)
                nc.vector.tensor_scalar(out=sh, in0=scor, scalar1=nmx,
                                        scalar2=-87.0, op0=OP.add, op1=OP.max)
                ex = small.tile([P, S], F32)
                es = small.tile([P, 1], F32)
                nc.scalar.activation(out=ex, in_=sh, func=AF.Exp,
                                     bias=0.0, scale=1.0, accum_out=es)
                ri = small.tile([P, 1], F32)
                nc.vector.reciprocal(out=ri, in_=es)
                at = small.tile([P, S], F32)
                nc.vector.tensor_scalar_mul(out=at, in0=ex, scalar1=ri)
                at4 = small.tile([S, NH, S], F32)
                for h in range(NH):
                    nc.sync.dma_start(out=at4[:, h, :], in_=at[h * S:(h + 1) * S, :])

                u2_ps = ps2.tile([C, C], F32, tag="mm")
                for h in range(NH):
                    nc.tensor.matmul(u2_ps[:, h * S:(h + 1) * S],
                                     lhsT=wvT_sb[:, h, :], rhs=at4[:, h, :],
                                     start=True, stop=True)
                u2_sb = small.tile([C, C], F32)
                nc.scalar.copy(out=u2_sb, in_=u2_ps)
                ut_ps = ps2.tile([C, C], F32, tag="mm")
                nc.tensor.transpose(ut_ps, u2_sb, id32_sb)
                ut_sb = small.tile([C, C], F32)
                nc.scalar.copy(out=ut_sb, in_=ut_ps)
                w2_ps = ps2.tile([C, C], F32, tag="mm")
                nc.tensor.matmul(w2_ps, lhsT=ut_sb, rhs=wf_sb, start=True, stop=True)
                w2_sb = small.tile([C, C], F16)
                nc.vector.tensor_tensor(out=w2_sb, in0=w2_ps, in1=dg_sb, op=OP.add)
                ws_ps = ps2.tile([1, C], F32, tag="sm")
                nc.tensor.matmul(ws_ps, lhsT=on16_sb[:, 0:1], rhs=w2_sb,
                                 start=True, stop=True)
                nws_sb = small.tile([1, C], F16)
                nc.vector.tensor_scalar_mul(out=nws_sb, in0=ws_ps, scalar1=-1.0)

            # ====== Phase 3: yp = W2^T (x*rb) - w2s (x) rmu; cast-DMA out ==
            with tc.tile_pool(name="psY", bufs=3, space="PSUM") as psY:
                for i in range(npa):
                    if i + npre < npa:
                        ii = i + npre
                        nc.sync.dma_start(
                            out=xt_tiles[ii],
                            in_=xt_in[:, 2 * ii * YC:(2 * ii + 2) * YC])
                    for k in range(2):
                        q = 2 * i + k
                        tsl = slice(q * YC, (q + 1) * YC)
                        rb_ps = psY.tile([C, YC], F32, tag="y")
                        nc.tensor.matmul(rb_ps, lhsT=on16_sb[0:1, :],
                                         rhs=r_row1[0:1, tsl],
                                         start=True, stop=True)
                        xts = sqbuf.tile([C, YC], F16, name="xts", tag="sq")
                        nc.vector.tensor_tensor(out=xts, in0=xt_tiles[i][:, k],
                                                in1=rb_ps, op=OP.mult)
                        yp = psY.tile([C, YC], F32, tag="y")
                        nc.tensor.matmul(yp, lhsT=w2_sb, rhs=xts,
                                         start=True, stop=False)
                        nc.tensor.matmul(yp, lhsT=nws_sb,
                                         rhs=rmu_row1[0:1, tsl],
                                         start=False, stop=True)
                        y16 = sqbuf.tile([C, YC], F16, name="y16", tag="y16")
                        nc.scalar.copy(out=y16, in_=yp)
                        nc.gpsimd.dma_start(out=yT_out[:, tsl], in_=y16)

    nc.compile()
    return nc


def _numpy_reference(x, gamma, beta, Wq, bq, Wk, bk, Wv, bv, Wf, bf, alpha):
    """Fallback for inputs outside the zero-bias fast path."""
    Bx, Hx, Wx, Cx = x.shape
    t = Hx * Wx
    nh = NH
    s = Cx // nh
    xf = x.reshape(Bx, t, Cx).astype(np.float64)
    mu = xf.mean(-1, keepdims=True)
    var = ((xf - mu) ** 2).mean(-1, keepdims=True)
    xn = (xf - mu) / np.sqrt(var + EPS) * gamma + beta
    Q = (xn @ Wq + bq).reshape(Bx, t, nh, s)
    K = (xn @ Wk + bk).reshape(Bx, t, nh, s)
    V = (xn @ Wv + bv).reshape(Bx, t, nh, s)
    scores = np.einsum("bthi,bthj->bhij", K, Q) / float(alpha)
    scores = scores - scores.max(-1, keepdims=True)
    e = np.exp(scores)
    attn = e / e.sum(-1, keepdims=True)
    out = np.einsum("bthi,bhij->bthj", V, attn).reshape(Bx, t, Cx)
    y = out @ Wf + bf + xn
    return y.reshape(Bx, Hx, Wx, Cx).astype(np.float32)


def make_in_maps(inputs, tloc=TLOC, n_cores=N_CORES):
    x = np.asarray(inputs["x"], dtype=np.float32)
    gamma = np.asarray(inputs["gamma"], dtype=np.float32)
    Wq = np.asarray(inputs["Wq"], dtype=np.float32)
    Wk = np.asarray(inputs["Wk"], dtype=np.float32)
    Wv = np.asarray(inputs["Wv"], dtype=np.float32)
    Wf = np.ascontiguousarray(np.asarray(inputs["Wf"], dtype=np.float32))
    inv_alpha = (1.0 / float(np.asarray(inputs["alpha"]))
                 if "alpha" in inputs else 1.0)

    wq_g = np.ascontiguousarray(gamma[:, None] * Wq * inv_alpha)
    wk_g = np.ascontiguousarray(gamma[:, None] * Wk)
    wv_g = gamma[:, None] * Wv
    wvT4 = np.ascontiguousarray(
        wv_g.T.reshape(NH, S, C).transpose(1, 0, 2).reshape(S, NH * C))
    diag_g = np.ascontiguousarray(np.diag(gamma).astype(np.float32))
    ident_f32 = np.eye(P, dtype=np.float32)
    ident_f16 = np.eye(P, dtype=np.float16)

    w1q = wq_g.sum(axis=0)
    w1q_pk = np.repeat(w1q.reshape(NH, S), S, axis=0).astype(np.float32)
    k1 = wk_g.sum(axis=0)
    k1_col = np.stack([k1, -k1], axis=1).astype(np.float32)
    hsel = (np.arange(C)[None, :] // S == np.arange(NH)[:, None]
            ).astype(np.float32)
    nyc = tloc // YC
    eqsel = np.zeros((P, 2 * nyc - 1), np.float16)
    eqsel[:, nyc - 1] = 1.0     # E_q = eqsel[:, nyc-1-q : 2*nyc-1-q]
    ones16 = np.ones((P, P), np.float16)
    ones32 = np.ones((P, P), np.float32)

    x16 = x.reshape(n_cores, tloc, C).astype(np.float16)
    ngrp = tloc // (P * GRP)
    x_nat = np.ascontiguousarray(
        x16.reshape(n_cores, ngrp, GRP, P, C).transpose(0, 1, 3, 2, 4)
        .reshape(n_cores, ngrp, P, GRP * C))
    x_tr = np.ascontiguousarray(x16.transpose(0, 2, 1))

    shared = dict(wq_g=wq_g, wk_g=wk_g, wvT4=wvT4, wf=Wf, diag_gamma=diag_g,
                  ident_f32=ident_f32, ident_f16=ident_f16,
                  w1q_pk=np.ascontiguousarray(w1q_pk),
                  k1_col=np.ascontiguousarray(k1_col),
                  hsel=np.ascontiguousarray(hsel), eqsel=eqsel,
                  ones16=ones16, ones32=ones32)
    return [dict(shared, x_nat=x_nat[i], x_tr=x_tr[i]) for i in range(n_cores)]


_NC_CACHE = {}


def kernel(**inputs) -> np.ndarray:
    zero = lambda k: not np.any(np.asarray(inputs[k]))
    if not (zero("beta") and zero("bq") and zero("bk") and zero("bv")
            and zero("bf")):
        return _numpy_reference(**{k: np.asarray(v) for k, v in inputs.items()})

    key = ("v3", TLOC, N_CORES)
    if key not in _NC_CACHE:
        _NC_CACHE[key] = build_nc(TLOC, N_CORES)
    nc = _NC_CACHE[key]

    in_maps = make_in_maps(inputs)
    res = run_bass_kernel_spmd(nc, in_maps, core_ids=list(range(N_CORES)))
    yT = [res.results[i]["yT16"] for i in range(N_CORES)]
    y = np.concatenate([t.T for t in yT], axis=0).astype(np.float32)
    return np.ascontiguousarray(y.reshape(B, HH, WW, C))



# revision 7
# speedup vs baseline: 1.3243x; 1.3243x over previous
"""MDTA (channel-attention transformer block) Trainium2 kernel, v4.

Math (zero-bias fast path; x16 = fp16(x), per-token mu/r from x16):
  G_needed = sum_t r^2 (x-mu*1)(x-mu*1)^T = G2 - u 1^T - 1 u^T + s 1 1^T
    G2 = sum r^2 x x^T = (r^2 x)^T X,  u = sum r^2 mu x,  s = 1^T u / C
  scores = wk'^T G wq'/alpha (diag 32x32 blocks), attn = softmax
  W2 = diag(g) Wv blockdiag(attn) Wf + diag(gamma),  w2s = 1^T W2
  y_t = r_t(W2^T x16_t) - r_t mu_t w2s = W2^T(x*rb) - w2s (x) rmu_row

Structure (v4 = v3 rescheduled around SBUF residency + overlap):
  Both layouts of x (xT [c, t] and x_nat [t, c]-grouped, host-staged fp16)
  are DMA'd ONCE into SBUF up front (16 MB resident) on the two HWDGE
  rings; a PE warmup burst at t=0 flips HAM to full clock.  Phase A
  (per-token sums via shifted-ones selector matmuls) runs on resident xT
  chunks as they land, in two 32-row halves so stats math / transposes /
  the Gram of half 0 overlap the stats of half 1.  The Gram rhs is the
  resident x_nat row with a 129th column that the kernel fills with mu,
  so u = sum r^2 mu x falls out of the same matmul (no N=1 matmuls).
  zr2 = x*r^2 scaling alternates DVE tensor_scalar / ACT mul (per-
  partition scalars in nat layout).  The pair all-reduce of [G2 | u]
  runs concurrently with all phase-3 prep: xts = xt * r is computed
  in place over the resident xT for all chunks (PE broadcast builds +
  DVE multiplies) while the collective is in flight.  Phase 3 then runs
  8 PSUM banks at a time (batched W2 / rank-1 weight loads), alternating
  ACT/DVE psum->fp16 copies, and writes yT out in 1 MB HWDGE DMAs.

Sharding: 8 cores = (batch 0..3) x (token half 0..1); 66 KB pair all-reduce.
Host does layout/dtype staging only (fp16 casts, the [c, t] transpose,
gamma/alpha folding, final yT.T -> fp32).
"""

import sys

import numpy as np

for _p in ("/opt/trn_rl_repo",):
    if _p not in sys.path:
        sys.path.append(_p)

import concourse.bacc as bacc
import concourse.bass as bass
import concourse.tile as tile
from concourse import mybir
from concourse.bass_utils import run_bass_kernel_spmd

B, HH, WW, C = 4, 256, 256, 128
NH, S = 4, 32
T = HH * WW
N_CORES = 8
TLOC = T // 2
EPS = 1e-5
P = 128
GRP = 4
YC = 512
C1 = C + 1          # x_nat row stride: C channels + mu slot

F32 = mybir.dt.float32
F16 = mybir.dt.float16

AF = mybir.ActivationFunctionType
OP = mybir.AluOpType
AX = mybir.AxisListType


def build_nc(tloc=TLOC, n_cores=N_CORES):
    assert tloc % (P * GRP) == 0 and tloc % YC == 0
    nc = bacc.Bacc("TRN2", target_bir_lowering=False, debug=False,
                   num_devices=n_cores)

    ngrp = tloc // (P * GRP)  # token groups of 512 (= chunks of 512)
    nyc = tloc // YC          # stats row count; == ngrp
    assert nyc == ngrp and nyc <= 64 and nyc % 2 == 0
    HQ = nyc // 2             # rows per stats half

    x_in = nc.declare_dram_parameter("x_nat", [ngrp, P, GRP * C1], F16,
                                     isOutput=False)
    xt_in = nc.declare_dram_parameter("x_tr", [C, tloc], F16, isOutput=False)
    wq_in = nc.declare_dram_parameter("wq_g", [C, C], F32, isOutput=False)
    wk_in = nc.declare_dram_parameter("wk_g", [C, C], F32, isOutput=False)
    wvT_in = nc.declare_dram_parameter("wvT4", [S, NH * C], F32, isOutput=False)
    wf_in = nc.declare_dram_parameter("wf", [C, C], F32, isOutput=False)
    dg_in = nc.declare_dram_parameter("diag_gamma", [C, C], F32, isOutput=False)
    id32_in = nc.declare_dram_parameter("ident_f32", [P, P], F32, isOutput=False)
    id16_in = nc.declare_dram_parameter("ident_f16", [P, P], F16, isOutput=False)
    w1q_in = nc.declare_dram_parameter("w1q_pk", [C, S], F32, isOutput=False)
    k1_in = nc.declare_dram_parameter("k1_col", [C, 2], F32, isOutput=False)
    hsel_in = nc.declare_dram_parameter("hsel", [NH, C], F32, isOutput=False)
    eq_in = nc.declare_dram_parameter("eqsel", [P, 2 * nyc - 1], F16,
                                      isOutput=False)
    on16_in = nc.declare_dram_parameter("ones16", [P, P], F16, isOutput=False)
    on32_in = nc.declare_dram_parameter("ones32", [P, P], F32, isOutput=False)
    yT_out = nc.declare_dram_parameter("yT16", [C, tloc], F16, isOutput=True)

    replica_groups = [[2 * b, 2 * b + 1] for b in range(n_cores // 2)]

    XDMA = 8                 # xT preload transfers (1 MB each)
    NDMA = 16                # x_nat preload transfers (~516 KB each)
    xtw = tloc // XDMA
    ndw = ngrp // NDMA

    with tile.TileContext(nc) as tc:
        with (
            tc.tile_pool(name="const", bufs=1) as const,
            tc.tile_pool(name="sqbuf", bufs=4) as sqbuf,
            tc.tile_pool(name="small", bufs=2) as small,
            tc.tile_pool(name="ybuf", bufs=2) as ybuf,
            tc.tile_pool(name="dram", bufs=1, space="DRAM") as dram,
        ):
            # ---- PE warmup burst (HAM -> full clock), no DMA deps ----
            wu_sb = const.tile([P, YC], F16)
            nc.vector.memset(wu_sb, 0.0)
            with tc.tile_pool(name="psW", bufs=1, space="PSUM") as psW:
                wu_ps = psW.tile([P, YC], F32, tag="wu")
                for _ in range(14):
                    nc.tensor.matmul(wu_ps, lhsT=wu_sb[:, 0:P], rhs=wu_sb,
                                     start=True, stop=True)

            # ---- resident x (both layouts) ----
            xt_res = const.tile([C, tloc], F16)
            nat = const.tile([P, ngrp, GRP * C1], F16)
            for d in range(XDMA):
                nc.sync.dma_start(
                    out=xt_res[:, d * xtw:(d + 1) * xtw],
                    in_=xt_in[:, d * xtw:(d + 1) * xtw])
            for d in range(NDMA):
                nc.scalar.dma_start(
                    out=nat[:, d * ndw:(d + 1) * ndw],
                    in_=x_in[d * ndw:(d + 1) * ndw].rearrange("g p x -> p g x"))

            # ---- constants (SWDGE ring; gpsimd idle until collective) ----
            wq_sb = const.tile([C, C], F32)
            wk_sb = const.tile([C, C], F32)
            wvT_sb = const.tile([S, NH, C], F32)
            wf_sb = const.tile([C, C], F32)
            dg_sb = const.tile([C, C], F32)
            id32_sb = const.tile([P, P], F32)
            id16_sb = const.tile([P, P], F16)
            w1q_sb = const.tile([C, S], F32)
            k1_sb = const.tile([C, 2], F32)
            hsel_sb = const.tile([NH, C], F32)
            eq_sb = const.tile([P, 2 * nyc - 1], F16)
            on16_sb = const.tile([P, P], F16)
            on32_sb = const.tile([P, P], F32)
            nc.gpsimd.dma_start(out=id16_sb, in_=id16_in[:])
            nc.gpsimd.dma_start(out=id32_sb, in_=id32_in[:])
            nc.gpsimd.dma_start(out=eq_sb, in_=eq_in[:])
            nc.gpsimd.dma_start(out=wq_sb, in_=wq_in[:])
            nc.gpsimd.dma_start(out=wk_sb, in_=wk_in[:])
            nc.gpsimd.dma_start(out=wvT_sb,
                                in_=wvT_in[:].rearrange("s (h c) -> s h c", h=NH))
            nc.gpsimd.dma_start(out=wf_sb, in_=wf_in[:])
            nc.gpsimd.dma_start(out=dg_sb, in_=dg_in[:])
            nc.gpsimd.dma_start(out=w1q_sb, in_=w1q_in[:])
            nc.gpsimd.dma_start(out=k1_sb, in_=k1_in[:])
            nc.gpsimd.dma_start(out=hsel_sb, in_=hsel_in[:])
            nc.gpsimd.dma_start(out=on16_sb, in_=on16_in[:])
            nc.gpsimd.dma_start(out=on32_sb, in_=on32_in[:])
            eps_sb = const.tile([P, 1], F32)
            nc.vector.memset(eps_sb, EPS)

            # stats row arrays [nyc, YC] (token t = 512*q + t')
            sx_sb = const.tile([nyc, YC], F32)
            sq_sb = const.tile([nyc, YC], F32)
            scr_sb = const.tile([nyc, YC], F32)
            scr2_sb = const.tile([nyc, YC], F32)
            mu16_sb = const.tile([nyc, YC], F16)
            rmu16_sb = const.tile([nyc, YC], F16)
            r16_sb = const.tile([nyc, YC], F16)
            # column-layout r^2 (fp32, per-partition scalars for zr2)
            r2col = const.tile([P, GRP, ngrp], F32)

            ZRING = 8
            zr2 = const.tile([P, ZRING, C], F16)
            g_sb = small.tile([C, C1], F32)

            with (
                tc.tile_pool(name="psS", bufs=2, space="PSUM") as psS,
                tc.tile_pool(name="ps2", bufs=1, space="PSUM") as ps2,
                tc.tile_pool(name="psG", bufs=1, space="PSUM") as psG,
            ):
                G_ps = psG.tile([C, C1], F32, tag="g")
                nlast = ngrp * GRP - 1
                for mh in range(2):
                    q0 = mh * HQ
                    # ======== Phase A (half mh): per-token sums via PE ====
                    sx_ps = psS.tile([HQ, YC], F32, tag="sx")
                    sq_ps = psS.tile([HQ, YC], F32, tag="sq")
                    for qq in range(HQ):
                        q = q0 + qq
                        xtq = xt_res[:, q * YC:(q + 1) * YC]
                        sqg = sqbuf.tile([C, YC], F16, name="sqg", tag="sq")
                        if q % 2 == 0:
                            nc.vector.tensor_tensor(out=sqg, in0=xtq, in1=xtq,
                                                    op=OP.mult)
                        else:
                            nc.scalar.square(out=sqg, in_=xtq)
                        c0 = nyc - 1 - q + q0
                        eq_v = eq_sb[:, c0:c0 + HQ]
                        nc.tensor.matmul(sx_ps, lhsT=eq_v, rhs=xtq,
                                         start=(qq == 0), stop=(qq == HQ - 1))
                        nc.tensor.matmul(sq_ps, lhsT=eq_v, rhs=sqg,
                                         start=(qq == 0), stop=(qq == HQ - 1))
                    sl = slice(q0, q0 + HQ)
                    nc.vector.tensor_copy(out=sx_sb[sl], in_=sx_ps)
                    nc.vector.tensor_copy(out=sq_sb[sl], in_=sq_ps)

                    # ---- batched stats math on [HQ, YC] ----
                    nc.vector.tensor_tensor(out=scr_sb[sl], in0=sx_sb[sl],
                                            in1=sx_sb[sl], op=OP.mult)
                    nc.vector.scalar_tensor_tensor(
                        out=scr2_sb[sl], in0=scr_sb[sl],
                        scalar=float(-1.0 / C), in1=sq_sb[sl],
                        op0=OP.mult, op1=OP.add)
                    nc.scalar.activation(out=scr_sb[sl], in_=scr2_sb[sl],
                                         func=AF.Sqrt, bias=eps_sb[0:HQ, :],
                                         scale=float(1.0 / C))
                    nc.vector.reciprocal(out=scr2_sb[sl], in_=scr_sb[sl])
                    nc.scalar.mul(out=mu16_sb[sl], in_=sx_sb[sl],
                                  mul=float(1.0 / C))
                    nc.vector.tensor_tensor(out=rmu16_sb[sl], in0=mu16_sb[sl],
                                            in1=scr2_sb[sl], op=OP.mult)
                    nc.scalar.copy(out=r16_sb[sl], in_=scr2_sb[sl])
                    nc.vector.tensor_tensor(out=scr_sb[sl], in0=scr2_sb[sl],
                                            in1=scr2_sb[sl], op=OP.mult)

                    # column layouts: r^2 -> r2col (fp32), mu -> nat mu slots
                    id_h32 = id32_sb[sl, sl]
                    id_h16 = id16_sb[sl, sl]
                    for j in range(GRP):
                        tpj = ps2.tile([P, HQ], F32, tag="tp")
                        nc.tensor.transpose(tpj, scr_sb[sl, j * P:(j + 1) * P],
                                            id_h32)
                        nc.scalar.copy(out=r2col[:, j, sl], in_=tpj)
                        tpm = ps2.tile([P, HQ], F16, tag="tp")
                        nc.tensor.transpose(tpm, mu16_sb[sl, j * P:(j + 1) * P],
                                            id_h16)
                        nc.scalar.copy(out=nat[:, sl, j * C1 + C], in_=tpm)

                    # ==== Gram for half mh: G2 += (r^2 x)^T [x | mu] ====
                    for g in range(q0, q0 + HQ):
                        for j in range(GRP):
                            i = g * GRP + j
                            r = i % ZRING
                            xnj = nat[:, g, j * C1:j * C1 + C]
                            if i % 2 == 0:
                                nc.vector.tensor_scalar_mul(
                                    out=zr2[:, r], in0=xnj,
                                    scalar1=r2col[:, j, g:g + 1])
                            else:
                                nc.scalar.mul(out=zr2[:, r], in_=xnj,
                                              mul=r2col[:, j, g:g + 1])
                            nc.tensor.matmul(G_ps, lhsT=zr2[:, r],
                                             rhs=nat[:, g, j * C1:(j + 1) * C1],
                                             start=(i == 0), stop=(i == nlast))

                nc.vector.tensor_copy(out=g_sb, in_=G_ps)

            # ============ all-reduce [G2 | u] ============
            g_in_d = dram.tile([C, C1], F32)
            g_out_d = dram.tile([C, C1], F32)
            nc.gpsimd.dma_start(out=g_in_d, in_=g_sb)
            nc.gpsimd.collective_compute(
                "AllReduce", OP.add, replica_groups=replica_groups,
                ins=[g_in_d[:].opt()], outs=[g_out_d[:].opt()])

            # -- overlap: xts = xt * r, in place over resident xT --
            RB = 8            # chunks per row-remap block
            with (
                tc.tile_pool(name="psR", bufs=4, space="PSUM") as psR,
                tc.tile_pool(name="rowp", bufs=2) as rowp,
            ):
                for b in range(nyc // RB):
                    rt = rowp.tile([1, RB * YC], F16, name="rt", tag="rt")
                    nc.sync.dma_start(out=rt,
                                      in_=r16_sb[b * RB:(b + 1) * RB, :])
                    for k in range(RB):
                        q = b * RB + k
                        tsl = slice(q * YC, (q + 1) * YC)
                        rb_ps = psR.tile([C, YC], F32, tag="rb")
                        nc.tensor.matmul(rb_ps, lhsT=on16_sb[0:1, :],
                                         rhs=rt[0:1, k * YC:(k + 1) * YC],
                                         start=True, stop=True)
                        nc.vector.tensor_tensor(out=xt_res[:, tsl],
                                                in0=xt_res[:, tsl],
                                                in1=rb_ps, op=OP.mult)

            gs_sb = small.tile([C, C1], F32)
            nc.gpsimd.dma_start(out=gs_sb, in_=g_out_d)

            # ============ Phase 2: scores + softmax + W2 ============
            with tc.tile_pool(name="ps3", bufs=1, space="PSUM") as ps2:
                u_ap = gs_sb[:, C:C + 1]
                s1_ps = ps2.tile([C, C], F32, tag="mm")
                nc.tensor.matmul(s1_ps, lhsT=gs_sb[:, 0:C], rhs=wq_sb,
                                 start=True, stop=True)   # G symmetric
                s1_sb = small.tile([C, C], F32)
                nc.scalar.copy(out=s1_sb, in_=s1_ps)
                sc_ps = ps2.tile([C, C], F32, tag="mm")
                nc.tensor.matmul(sc_ps, lhsT=wk_sb, rhs=s1_sb, start=True,
                                 stop=True)
                spk = small.tile([P, S], F32)
                for h in range(NH):
                    nc.scalar.copy(out=spk[h * S:(h + 1) * S, :],
                                   in_=sc_ps[h * S:(h + 1) * S,
                                             h * S:(h + 1) * S])

                a_ps = ps2.tile([C, 1], F32, tag="sm")
                nc.tensor.matmul(a_ps, lhsT=wk_sb, rhs=u_ap, start=True,
                                 stop=True)
                a_sb = small.tile([C, 1], F32)
                nc.vector.tensor_copy(out=a_sb, in_=a_ps)
                bc_ps = ps2.tile([C, 1], F32, tag="sm")
                nc.tensor.matmul(bc_ps, lhsT=wq_sb, rhs=u_ap, start=True,
                                 stop=True)
                bc_sb = small.tile([C, 1], F32)
                nc.scalar.copy(out=bc_sb, in_=bc_ps)
                su_ps = ps2.tile([1, 1], F32, tag="sm")
                nc.tensor.matmul(su_ps, lhsT=u_ap, rhs=on32_sb[:, 0:1],
                                 start=True, stop=True)
                su_sb = small.tile([1, 1], F32)
                nc.scalar.copy(out=su_sb, in_=su_ps)
                sc_col_ps = ps2.tile([C, 1], F32, tag="sm")
                nc.tensor.matmul(sc_col_ps, lhsT=on32_sb[0:1, :], rhs=su_sb,
                                 start=True, stop=True)
                scol_sb = small.tile([C, 1], F32)
                nc.scalar.mul(out=scol_sb, in_=sc_col_ps, mul=float(1.0 / C))

                bT_ps = ps2.tile([1, C], F32, tag="sm")
                nc.tensor.transpose(bT_ps, bc_sb, id32_sb)
                bT_sb = small.tile([1, C], F32)
                nc.scalar.copy(out=bT_sb, in_=bT_ps)
                bT4_sb = small.tile([NH, S], F32)
                nc.sync.dma_start(out=bT4_sb, in_=bT_sb)
                bpk_ps = ps2.tile([C, S], F32, tag="sm")
                nc.tensor.matmul(bpk_ps, lhsT=hsel_sb, rhs=bT4_sb,
                                 start=True, stop=True)

                tmp_sb = small.tile([C, 1], F32)
                nc.vector.scalar_tensor_tensor(
                    out=tmp_sb, in0=scol_sb, scalar=k1_sb[:, 0:1], in1=a_sb,
                    op0=OP.mult, op1=OP.subtract)             # s*k1 - a
                s1c = small.tile([P, S], F32)
                nc.vector.scalar_tensor_tensor(
                    out=s1c, in0=w1q_sb, scalar=tmp_sb, in1=spk,
                    op0=OP.mult, op1=OP.add)
                scor = small.tile([P, S], F32)
                nc.vector.scalar_tensor_tensor(
                    out=scor, in0=bpk_ps, scalar=k1_sb[:, 1:2], in1=s1c,
                    op0=OP.mult, op1=OP.add)

                mx = small.tile([P, 1], F32)
                nc.vector.reduce_max(mx, scor, AX.X)
                nmx = small.tile([P, 1], F32)
                nc.vector.tensor_scalar_mul(out=nmx, in0=mx, scalar1=-1.0)
                sh = small.tile([P, S], F32)
                nc.vector.tensor_scalar(out=sh, in0=scor, scalar1=nmx,
                                        scalar2=-87.0, op0=OP.add, op1=OP.max)
                ex = small.tile([P, S], F32)
                es = small.tile([P, 1], F32)
                nc.scalar.activation(out=ex, in_=sh, func=AF.Exp,
                                     bias=0.0, scale=1.0, accum_out=es)
                ri = small.tile([P, 1], F32)
                nc.vector.reciprocal(out=ri, in_=es)
                at = small.tile([P, S], F32)
                nc.vector.tensor_scalar_mul(out=at, in0=ex, scalar1=ri)
                at4 = small.tile([S, NH, S], F32)
                for h in range(NH):
                    nc.sync.dma_start(out=at4[:, h, :],
                                      in_=at[h * S:(h + 1) * S, :])

                u2_ps = ps2.tile([C, C], F32, tag="mm")
                for h in range(NH):
                    nc.tensor.matmul(u2_ps[:, h * S:(h + 1) * S],
                                     lhsT=wvT_sb[:, h, :], rhs=at4[:, h, :],
                                     start=True, stop=True)
                u2_sb = small.tile([C, C], F32)
                nc.scalar.copy(out=u2_sb, in_=u2_ps)
                ut_ps = ps2.tile([C, C], F32, tag="mm")
                nc.tensor.transpose(ut_ps, u2_sb, id32_sb)
                ut_sb = small.tile([C, C], F32)
                nc.scalar.copy(out=ut_sb, in_=ut_ps)
                w2_ps = ps2.tile([C, C], F32, tag="mm")
                nc.tensor.matmul(w2_ps, lhsT=ut_sb, rhs=wf_sb, start=True,
                                 stop=True)
                w2_sb = small.tile([C, C], F16)
                nc.vector.tensor_tensor(out=w2_sb, in0=w2_ps, in1=dg_sb,
                                        op=OP.add)
                ws_ps = ps2.tile([1, C], F32, tag="sm")
                nc.tensor.matmul(ws_ps, lhsT=on16_sb[:, 0:1], rhs=w2_sb,
                                 start=True, stop=True)
                nws_sb = small.tile([1, C], F16)
                nc.vector.tensor_scalar_mul(out=nws_sb, in0=ws_ps, scalar1=-1.0)

            # ====== Phase 3: yp = W2^T xts - w2s (x) rmu; fp16 out ======
            YB = 8            # PSUM banks per block
            nyb = nyc // YB
            with (
                tc.tile_pool(name="psY", bufs=YB, space="PSUM") as psY,
                tc.tile_pool(name="rowq", bufs=2) as rowq,
            ):
                for blk in range(nyb):
                    rmt = rowq.tile([1, YB * YC], F16, name="rmt", tag="rmt")
                    nc.sync.dma_start(
                        out=rmt, in_=rmu16_sb[blk * YB:(blk + 1) * YB, :])
                    yps = []
                    for k in range(YB):
                        q = blk * YB + k
                        tsl = slice(q * YC, (q + 1) * YC)
                        yp = psY.tile([C, YC], F32, tag="y")
                        nc.tensor.matmul(yp, lhsT=w2_sb, rhs=xt_res[:, tsl],
                                         start=True, stop=False)
                        yps.append(yp)
                    for k in range(YB):
                        nc.tensor.matmul(yps[k], lhsT=nws_sb,
                                         rhs=rmt[0:1, k * YC:(k + 1) * YC],
                                         start=False, stop=True)
                    y16 = ybuf.tile([C, YB * YC], F16, name="y16", tag="y16")
                    for k in range(YB):
                        if k % 2 == 0:
                            nc.scalar.copy(out=y16[:, k * YC:(k + 1) * YC],
                                           in_=yps[k])
                        else:
                            nc.vector.tensor_copy(
                                out=y16[:, k * YC:(k + 1) * YC], in_=yps[k])
                    osl = slice(blk * YB * YC, (blk + 1) * YB * YC)
                    nc.sync.dma_start(out=yT_out[:, osl], in_=y16)

    nc.compile()
    return nc


def _numpy_reference(x, gamma, beta, Wq, bq, Wk, bk, Wv, bv, Wf, bf, alpha):
    """Fallback for inputs outside the zero-bias fast path."""
    Bx, Hx, Wx, Cx = x.shape
    t = Hx * Wx
    nh = NH
    s = Cx // nh
    xf = x.reshape(Bx, t, Cx).astype(np.float64)
    mu = xf.mean(-1, keepdims=True)
    var = ((xf - mu) ** 2).mean(-1, keepdims=True)
    xn = (xf - mu) / np.sqrt(var + EPS) * gamma + beta
    Q = (xn @ Wq + bq).reshape(Bx, t, nh, s)
    K = (xn @ Wk + bk).reshape(Bx, t, nh, s)
    V = (xn @ Wv + bv).reshape(Bx, t, nh, s)
    scores = np.einsum("bthi,bthj->bhij", K, Q) / float(alpha)
    scores = scores - scores.max(-1, keepdims=True)
    e = np.exp(scores)
    attn = e / e.sum(-1, keepdims=True)
    out = np.einsum("bthi,bhij->bthj", V, attn).reshape(Bx, t, Cx)
    y = out @ Wf + bf + xn
    return y.reshape(Bx, Hx, Wx, Cx).astype(np.float32)


def make_in_maps(inputs, tloc=TLOC, n_cores=N_CORES):
    x = np.asarray(inputs["x"], dtype=np.float32)
    gamma = np.asarray(inputs["gamma"], dtype=np.float32)
    Wq = np.asarray(inputs["Wq"], dtype=np.float32)
    Wk = np.asarray(inputs["Wk"], dtype=np.float32)
    Wv = np.asarray(inputs["Wv"], dtype=np.float32)
    Wf = np.ascontiguousarray(np.asarray(inputs["Wf"], dtype=np.float32))
    inv_alpha = (1.0 / float(np.asarray(inputs["alpha"]))
                 if "alpha" in inputs else 1.0)

    wq_g = np.ascontiguousarray(gamma[:, None] * Wq * inv_alpha)
    wk_g = np.ascontiguousarray(gamma[:, None] * Wk)
    wv_g = gamma[:, None] * Wv
    wvT4 = np.ascontiguousarray(
        wv_g.T.reshape(NH, S, C).transpose(1, 0, 2).reshape(S, NH * C))
    diag_g = np.ascontiguousarray(np.diag(gamma).astype(np.float32))
    ident_f32 = np.eye(P, dtype=np.float32)
    ident_f16 = np.eye(P, dtype=np.float16)

    w1q = wq_g.sum(axis=0)
    w1q_pk = np.repeat(w1q.reshape(NH, S), S, axis=0).astype(np.float32)
    k1 = wk_g.sum(axis=0)
    k1_col = np.stack([k1, -k1], axis=1).astype(np.float32)
    hsel = (np.arange(C)[None, :] // S == np.arange(NH)[:, None]
            ).astype(np.float32)
    nyc = tloc // YC
    eqsel = np.zeros((P, 2 * nyc - 1), np.float16)
    eqsel[:, nyc - 1] = 1.0     # E_q = eqsel[:, nyc-1-q : 2*nyc-1-q]
    ones16 = np.ones((P, P), np.float16)
    ones32 = np.ones((P, P), np.float32)

    x16 = x.reshape(n_cores, tloc, C).astype(np.float16)
    ngrp = tloc // (P * GRP)
    # x_nat with a zero 129th column per token (kernel fills it with mu)
    xg = x16.reshape(n_cores, ngrp, GRP, P, C).transpose(0, 1, 3, 2, 4)
    xpad = np.zeros((n_cores, ngrp, P, GRP, C1), np.float16)
    xpad[..., :C] = xg
    x_nat = np.ascontiguousarray(
        xpad.reshape(n_cores, ngrp, P, GRP * C1))
    x_tr = np.ascontiguousarray(x16.transpose(0, 2, 1))

    shared = dict(wq_g=wq_g, wk_g=wk_g, wvT4=wvT4, wf=Wf, diag_gamma=diag_g,
                  ident_f32=ident_f32, ident_f16=ident_f16,
                  w1q_pk=np.ascontiguousarray(w1q_pk),
                  k1_col=np.ascontiguousarray(k1_col),
                  hsel=np.ascontiguousarray(hsel), eqsel=eqsel,
                  ones16=ones16, ones32=ones32)
    return [dict(shared, x_nat=x_nat[i], x_tr=x_tr[i]) for i in range(n_cores)]


_NC_CACHE = {}


def kernel(**inputs) -> np.ndarray:
    zero = lambda k: not np.any(np.asarray(inputs[k]))
    if not (zero("beta") and zero("bq") and zero("bk") and zero("bv")
            and zero("bf")):
        return _numpy_reference(**{k: np.asarray(v) for k, v in inputs.items()})

    key = ("v4", TLOC, N_CORES)
    if key not in _NC_CACHE:
        _NC_CACHE[key] = build_nc(TLOC, N_CORES)
    nc = _NC_CACHE[key]

    in_maps = make_in_maps(inputs)
    res = run_bass_kernel_spmd(nc, in_maps, core_ids=list(range(N_CORES)))
    yT = [res.results[i]["yT16"] for i in range(N_CORES)]
    y = np.concatenate([t.T for t in yT], axis=0).astype(np.float32)
    return np.ascontiguousarray(y.reshape(B, HH, WW, C))


# revision 14
# speedup vs baseline: 1.4462x; 1.0921x over previous
"""MDTA (channel-attention transformer block) Trainium2 kernel, v4.

Math (zero-bias fast path; x16 = fp16(x), per-token mu/r from x16):
  G_needed = sum_t r^2 (x-mu*1)(x-mu*1)^T = G2 - u 1^T - 1 u^T + s 1 1^T
    G2 = sum r^2 x x^T = (r^2 x)^T X,  u = sum r^2 mu x,  s = 1^T u / C
  scores = wk'^T G wq'/alpha (diag 32x32 blocks), attn = softmax
  W2 = diag(g) Wv blockdiag(attn) Wf + diag(gamma),  w2s = 1^T W2
  y_t = r_t(W2^T x16_t) - r_t mu_t w2s = W2^T(x*rb) - w2s (x) rmu_row

Structure (v4 = v3 rescheduled around SBUF residency + overlap):
  Both layouts of x (xT [c, t] and x_nat [t, c]-grouped, host-staged fp16)
  are DMA'd ONCE into SBUF up front (16 MB resident) on the two HWDGE
  rings; a PE warmup burst at t=0 flips HAM to full clock.  Phase A
  (per-token sums via shifted-ones selector matmuls) runs on resident xT
  chunks as they land, in two 32-row halves so stats math / transposes /
  the Gram of half 0 overlap the stats of half 1.  The Gram rhs is the
  resident x_nat row with a 129th column that the kernel fills with mu,
  so u = sum r^2 mu x falls out of the same matmul (no N=1 matmuls).
  zr2 = x*r^2 scaling alternates DVE tensor_scalar / ACT mul (per-
  partition scalars in nat layout).  The pair all-reduce of [G2 | u]
  runs concurrently with all phase-3 prep: xts = xt * r is computed
  in place over the resident xT for all chunks (PE broadcast builds +
  DVE multiplies) while the collective is in flight.  Phase 3 then runs
  8 PSUM banks at a time (batched W2 / rank-1 weight loads), alternating
  ACT/DVE psum->fp16 copies, and writes yT out in 1 MB HWDGE DMAs.

Sharding: 8 cores = (batch 0..3) x (token half 0..1); 66 KB pair all-reduce.
Host does layout/dtype staging only (fp16 casts, the [c, t] transpose,
gamma/alpha folding, final yT.T -> fp32).
"""

import sys

import numpy as np

for _p in ("/opt/trn_rl_repo",):
    if _p not in sys.path:
        sys.path.append(_p)

import concourse.bacc as bacc
import concourse.bass as bass
import concourse.tile as tile
from concourse import mybir
from concourse.bass_utils import run_bass_kernel_spmd

B, HH, WW, C = 4, 256, 256, 128
NH, S = 4, 32
T = HH * WW
N_CORES = 8
TLOC = T // 2
EPS = 1e-5
P = 128
GRP = 4
YC = 512
C1 = C + 1          # x_nat row stride: C channels + mu slot

F32 = mybir.dt.float32
F16 = mybir.dt.float16

AF = mybir.ActivationFunctionType
OP = mybir.AluOpType
AX = mybir.AxisListType


def build_nc(tloc=TLOC, n_cores=N_CORES):
    assert tloc % (P * GRP) == 0 and tloc % YC == 0
    nc = bacc.Bacc("TRN2", target_bir_lowering=False, debug=False,
                   num_devices=n_cores)

    ngrp = tloc // (P * GRP)  # token groups of 512 (= chunks of 512)
    nyc = tloc // YC          # stats row count; == ngrp
    assert nyc == ngrp and nyc <= 64 and nyc % 2 == 0
    HQ = nyc // 2             # rows per stats half

    x_in = nc.declare_dram_parameter("x_nat", [P, ngrp * GRP * C1], F16,
                                     isOutput=False)
    xt_in = nc.declare_dram_parameter("x_tr", [C, tloc], F16, isOutput=False)
    wq_in = nc.declare_dram_parameter("wq_g", [C, C], F32, isOutput=False)
    wk_in = nc.declare_dram_parameter("wk_g", [C, C], F32, isOutput=False)
    wvT_in = nc.declare_dram_parameter("wvT4", [S, NH * C], F32, isOutput=False)
    wf_in = nc.declare_dram_parameter("wf", [C, C], F32, isOutput=False)
    dg_in = nc.declare_dram_parameter("diag_gamma", [C, C], F32, isOutput=False)
    id32_in = nc.declare_dram_parameter("ident_f32", [P, P], F32, isOutput=False)
    id16_in = nc.declare_dram_parameter("ident_f16", [P, P], F16, isOutput=False)
    w1q_in = nc.declare_dram_parameter("w1q_pk", [C, S], F32, isOutput=False)
    k1_in = nc.declare_dram_parameter("k1_col", [C, 2], F32, isOutput=False)
    hsel_in = nc.declare_dram_parameter("hsel", [NH, C], F32, isOutput=False)
    eq_in = nc.declare_dram_parameter("eqsel", [P, 2 * nyc - 1], F16,
                                      isOutput=False)
    on16_in = nc.declare_dram_parameter("ones16", [P, P], F16, isOutput=False)
    on32_in = nc.declare_dram_parameter("ones32", [P, P], F32, isOutput=False)
    yT_out = nc.declare_dram_parameter("yT16", [C, tloc], F16, isOutput=True)

    replica_groups = [[2 * b, 2 * b + 1] for b in range(n_cores // 2)]

    XDMA = 8                 # xT preload transfers (1 MB each)
    NDMA = 16                # x_nat preload transfers (~516 KB each)
    xtw = tloc // XDMA
    ndw = ngrp // NDMA

    with tile.TileContext(nc) as tc:
        with (
            tc.tile_pool(name="const", bufs=1) as const,
            tc.tile_pool(name="sqbuf", bufs=4) as sqbuf,
            tc.tile_pool(name="small", bufs=2) as small,
            tc.tile_pool(name="ybuf", bufs=2) as ybuf,
            tc.tile_pool(name="dram", bufs=1, space="DRAM") as dram,
        ):
            # ---- PE warmup burst (HAM -> full clock), no DMA deps ----
            wu_sb = const.tile([P, YC], F16)
            nc.vector.memset(wu_sb, 0.0)
            with tc.tile_pool(name="psW", bufs=1, space="PSUM") as psW:
                wu_ps = psW.tile([P, YC], F32, tag="wu")
                for _ in range(14):
                    nc.tensor.matmul(wu_ps, lhsT=wu_sb[:, 0:P], rhs=wu_sb,
                                     start=True, stop=True)

            # ---- resident x (both layouts) ----
            xt_res = const.tile([C, tloc], F16)
            nat = const.tile([P, ngrp, GRP * C1], F16)
            natf = nat[:].rearrange("p g x -> p (g x)")
            for d in range(XDMA):
                nc.sync.dma_start(
                    out=xt_res[:, d * xtw:(d + 1) * xtw],
                    in_=xt_in[:, d * xtw:(d + 1) * xtw])
            # ---- constants (SWDGE ring; gpsimd idle until collective) ----
            wq_sb = const.tile([C, C], F32)
            wk_sb = const.tile([C, C], F32)
            wvT_sb = const.tile([S, NH, C], F32)
            wf_sb = const.tile([C, C], F32)
            dg_sb = const.tile([C, C], F32)
            id32_sb = const.tile([P, P], F32)
            id16_sb = const.tile([P, P], F16)
            w1q_sb = const.tile([C, S], F32)
            k1_sb = const.tile([C, 2], F32)
            hsel_sb = const.tile([NH, C], F32)
            eq_sb = const.tile([P, 2 * nyc - 1], F16)
            on16_sb = const.tile([P, P], F16)
            on32_sb = const.tile([P, P], F32)
            nc.gpsimd.dma_start(out=id16_sb, in_=id16_in[:])
            nc.gpsimd.dma_start(out=id32_sb, in_=id32_in[:])
            nc.gpsimd.dma_start(out=eq_sb, in_=eq_in[:])
            nc.gpsimd.dma_start(out=wq_sb, in_=wq_in[:])
            nc.gpsimd.dma_start(out=wk_sb, in_=wk_in[:])
            nc.gpsimd.dma_start(out=wvT_sb,
                                in_=wvT_in[:].rearrange("s (h c) -> s h c", h=NH))
            nc.gpsimd.dma_start(out=wf_sb, in_=wf_in[:])
            nc.gpsimd.dma_start(out=dg_sb, in_=dg_in[:])
            nc.gpsimd.dma_start(out=w1q_sb, in_=w1q_in[:])
            nc.gpsimd.dma_start(out=k1_sb, in_=k1_in[:])
            nc.gpsimd.dma_start(out=hsel_sb, in_=hsel_in[:])
            nc.gpsimd.dma_start(out=on16_sb, in_=on16_in[:])
            nc.gpsimd.dma_start(out=on32_sb, in_=on32_in[:])
            eps_sb = const.tile([P, 1], F32)
            nc.vector.memset(eps_sb, EPS)

            nw = ngrp * GRP * C1 // NDMA
            for d in range(NDMA):
                nc.gpsimd.dma_start(out=natf[:, d * nw:(d + 1) * nw],
                                    in_=x_in[:, d * nw:(d + 1) * nw])

            # stats row arrays [nyc, YC] (token t = 512*q + t')
            sx_sb = const.tile([nyc, YC], F32)
            sq_sb = const.tile([nyc, YC], F32)
            scr_sb = const.tile([nyc, YC], F32)
            scr2_sb = const.tile([nyc, YC], F32)
            mu16_sb = const.tile([nyc, YC], F16)
            rmu16_sb = const.tile([nyc, YC], F16)
            r16_sb = const.tile([nyc, YC], F16)
            # column-layout r^2 (fp32, per-partition scalars for zr2)
            r2col = const.tile([P, GRP, ngrp], F32)

            ZRING = 8
            zr2 = const.tile([P, ZRING, C], F16)
            g_sb = small.tile([C, C1], F32)

            with (
                tc.tile_pool(name="psS", bufs=2, space="PSUM") as psS,
                tc.tile_pool(name="ps2", bufs=1, space="PSUM") as ps2,
                tc.tile_pool(name="psG", bufs=1, space="PSUM") as psG,
            ):
                G_ps = psG.tile([C, C1], F32, tag="g")
                nlast = ngrp * GRP - 1
                for mh in range(2):
                    q0 = mh * HQ
                    # ======== Phase A (half mh): per-token sums via PE ====
                    sx_ps = psS.tile([HQ, YC], F32, tag="sx")
                    sq_ps = psS.tile([HQ, YC], F32, tag="sq")
                    for qq in range(HQ):
                        q = q0 + qq
                        xtq = xt_res[:, q * YC:(q + 1) * YC]
                        sqg = sqbuf.tile([C, YC], F16, name="sqg", tag="sq")
                        if q % 2 == 0:
                            nc.vector.tensor_tensor(out=sqg, in0=xtq, in1=xtq,
                                                    op=OP.mult)
                        else:
                            nc.scalar.square(out=sqg, in_=xtq)
                        c0 = nyc - 1 - q + q0
                        eq_v = eq_sb[:, c0:c0 + HQ]
                        nc.tensor.matmul(sx_ps, lhsT=eq_v, rhs=xtq,
                                         start=(qq == 0), stop=(qq == HQ - 1))
                        nc.tensor.matmul(sq_ps, lhsT=eq_v, rhs=sqg,
                                         start=(qq == 0), stop=(qq == HQ - 1))
                    sl = slice(q0, q0 + HQ)
                    nc.vector.tensor_copy(out=sx_sb[sl], in_=sx_ps)
                    nc.vector.tensor_copy(out=sq_sb[sl], in_=sq_ps)

                    # ---- batched stats math on [HQ, YC] ----
                    nc.vector.tensor_tensor(out=scr_sb[sl], in0=sx_sb[sl],
                                            in1=sx_sb[sl], op=OP.mult)
                    nc.vector.scalar_tensor_tensor(
                        out=scr2_sb[sl], in0=scr_sb[sl],
                        scalar=float(-1.0 / C), in1=sq_sb[sl],
                        op0=OP.mult, op1=OP.add)
                    nc.scalar.activation(out=scr_sb[sl], in_=scr2_sb[sl],
                                         func=AF.Sqrt, bias=eps_sb[0:HQ, :],
                                         scale=float(1.0 / C))
                    nc.vector.reciprocal(out=scr2_sb[sl], in_=scr_sb[sl])
                    nc.scalar.mul(out=mu16_sb[sl], in_=sx_sb[sl],
                                  mul=float(1.0 / C))
                    nc.vector.tensor_tensor(out=rmu16_sb[sl], in0=mu16_sb[sl],
                                            in1=scr2_sb[sl], op=OP.mult)
                    nc.scalar.copy(out=r16_sb[sl], in_=scr2_sb[sl])
                    nc.vector.tensor_tensor(out=scr_sb[sl], in0=scr2_sb[sl],
                                            in1=scr2_sb[sl], op=OP.mult)

                    # column layouts: r^2 -> r2col (fp32), mu -> nat mu slots
                    id_h32 = id32_sb[sl, sl]
                    id_h16 = id16_sb[sl, sl]
                    for j in range(GRP):
                        tpj = ps2.tile([P, HQ], F32, tag="tp")
                        nc.tensor.transpose(tpj, scr_sb[sl, j * P:(j + 1) * P],
                                            id_h32)
                        nc.scalar.copy(out=r2col[:, j, sl], in_=tpj)
                        tpm = ps2.tile([P, HQ], F16, tag="tp")
                        nc.tensor.transpose(tpm, mu16_sb[sl, j * P:(j + 1) * P],
                                            id_h16)
                        nc.scalar.copy(out=nat[:, sl, j * C1 + C], in_=tpm)

                    # ==== Gram for half mh: G2 += (r^2 x)^T [x | mu] ====
                    for g in range(q0, q0 + HQ):
                        for j in range(GRP):
                            i = g * GRP + j
                            r = i % ZRING
                            xnj = nat[:, g, j * C1:j * C1 + C]
                            if i % 2 == 0:
                                nc.vector.tensor_scalar_mul(
                                    out=zr2[:, r], in0=xnj,
                                    scalar1=r2col[:, j, g:g + 1])
                            else:
                                nc.scalar.mul(out=zr2[:, r], in_=xnj,
                                              mul=r2col[:, j, g:g + 1])
                            nc.tensor.matmul(G_ps, lhsT=zr2[:, r],
                                             rhs=nat[:, g, j * C1:(j + 1) * C1],
                                             start=(i == 0), stop=(i == nlast))

                nc.vector.tensor_copy(out=g_sb, in_=G_ps)

            # ============ all-reduce [G2 | u] ============
            g_in_d = dram.tile([C, C1], F32)
            g_out_d = dram.tile([C, C1], F32)
            nc.gpsimd.dma_start(out=g_in_d, in_=g_sb)
            nc.gpsimd.collective_compute(
                "AllReduce", OP.add, replica_groups=replica_groups,
                ins=[g_in_d[:].opt()], outs=[g_out_d[:].opt()])

            # -- overlap: xts = xt * r, in place over resident xT --
            RB = 8            # chunks per row-remap block
            with (
                tc.tile_pool(name="psR", bufs=4, space="PSUM") as psR,
                tc.tile_pool(name="rowp", bufs=2) as rowp,
            ):
                for b in range(nyc // RB):
                    rt = rowp.tile([1, RB * YC], F16, name="rt", tag="rt")
                    nc.sync.dma_start(out=rt,
                                      in_=r16_sb[b * RB:(b + 1) * RB, :])
                    for k in range(RB):
                        q = b * RB + k
                        tsl = slice(q * YC, (q + 1) * YC)
                        rb_ps = psR.tile([C, YC], F32, tag="rb")
                        nc.tensor.matmul(rb_ps, lhsT=on16_sb[0:1, :],
                                         rhs=rt[0:1, k * YC:(k + 1) * YC],
                                         start=True, stop=True)
                        rb16 = sqbuf.tile([C, YC], F16, name="rb16", tag="rb")
                        nc.scalar.copy(out=rb16, in_=rb_ps)
                        nc.vector.tensor_tensor(out=xt_res[:, tsl],
                                                in0=xt_res[:, tsl],
                                                in1=rb16, op=OP.mult)

            gs_sb = small.tile([C, C1], F32)
            nc.gpsimd.dma_start(out=gs_sb, in_=g_out_d)

            # ============ Phase 2: scores + softmax + W2 ============
            with tc.tile_pool(name="ps3", bufs=1, space="PSUM") as ps2:
                u_ap = gs_sb[:, C:C + 1]
                s1_ps = ps2.tile([C, C], F32, tag="mm")
                nc.tensor.matmul(s1_ps, lhsT=gs_sb[:, 0:C], rhs=wq_sb,
                                 start=True, stop=True)   # G symmetric
                s1_sb = small.tile([C, C], F32)
                nc.scalar.copy(out=s1_sb, in_=s1_ps)
                sc_ps = ps2.tile([C, C], F32, tag="mm")
                nc.tensor.matmul(sc_ps, lhsT=wk_sb, rhs=s1_sb, start=True,
                                 stop=True)
                spk = small.tile([P, S], F32)
                for h in range(NH):
                    nc.scalar.copy(out=spk[h * S:(h + 1) * S, :],
                                   in_=sc_ps[h * S:(h + 1) * S,
                                             h * S:(h + 1) * S])

                a_ps = ps2.tile([C, 1], F32, tag="sm")
                nc.tensor.matmul(a_ps, lhsT=wk_sb, rhs=u_ap, start=True,
                                 stop=True)
                a_sb = small.tile([C, 1], F32)
                nc.vector.tensor_copy(out=a_sb, in_=a_ps)
                bc_ps = ps2.tile([C, 1], F32, tag="sm")
                nc.tensor.matmul(bc_ps, lhsT=wq_sb, rhs=u_ap, start=True,
                                 stop=True)
                bc_sb = small.tile([C, 1], F32)
                nc.scalar.copy(out=bc_sb, in_=bc_ps)
                su_ps = ps2.tile([1, 1], F32, tag="sm")
                nc.tensor.matmul(su_ps, lhsT=u_ap, rhs=on32_sb[:, 0:1],
                                 start=True, stop=True)
                su_sb = small.tile([1, 1], F32)
                nc.scalar.copy(out=su_sb, in_=su_ps)
                sc_col_ps = ps2.tile([C, 1], F32, tag="sm")
                nc.tensor.matmul(sc_col_ps, lhsT=on32_sb[0:1, :], rhs=su_sb,
                                 start=True, stop=True)
                scol_sb = small.tile([C, 1], F32)
                nc.scalar.mul(out=scol_sb, in_=sc_col_ps, mul=float(1.0 / C))

                bT_ps = ps2.tile([1, C], F32, tag="sm")
                nc.tensor.transpose(bT_ps, bc_sb, id32_sb)
                bT_sb = small.tile([1, C], F32)
                nc.scalar.copy(out=bT_sb, in_=bT_ps)
                bT4_sb = small.tile([NH, S], F32)
                nc.sync.dma_start(out=bT4_sb, in_=bT_sb)
                bpk_ps = ps2.tile([C, S], F32, tag="sm")
                nc.tensor.matmul(bpk_ps, lhsT=hsel_sb, rhs=bT4_sb,
                                 start=True, stop=True)

                tmp_sb = small.tile([C, 1], F32)
                nc.vector.scalar_tensor_tensor(
                    out=tmp_sb, in0=scol_sb, scalar=k1_sb[:, 0:1], in1=a_sb,
                    op0=OP.mult, op1=OP.subtract)             # s*k1 - a
                s1c = small.tile([P, S], F32)
                nc.vector.scalar_tensor_tensor(
                    out=s1c, in0=w1q_sb, scalar=tmp_sb, in1=spk,
                    op0=OP.mult, op1=OP.add)
                scor = small.tile([P, S], F32)
                nc.vector.scalar_tensor_tensor(
                    out=scor, in0=bpk_ps, scalar=k1_sb[:, 1:2], in1=s1c,
                    op0=OP.mult, op1=OP.add)

                mx = small.tile([P, 1], F32)
                nc.vector.reduce_max(mx, scor, AX.X)
                nmx = small.tile([P, 1], F32)
                nc.vector.tensor_scalar_mul(out=nmx, in0=mx, scalar1=-1.0)
                sh = small.tile([P, S], F32)
                nc.vector.tensor_scalar(out=sh, in0=scor, scalar1=nmx,
                                        scalar2=-87.0, op0=OP.add, op1=OP.max)
                ex = small.tile([P, S], F32)
                es = small.tile([P, 1], F32)
                nc.scalar.activation(out=ex, in_=sh, func=AF.Exp,
                                     bias=0.0, scale=1.0, accum_out=es)
                ri = small.tile([P, 1], F32)
                nc.vector.reciprocal(out=ri, in_=es)
                at = small.tile([P, S], F32)
                nc.vector.tensor_scalar_mul(out=at, in0=ex, scalar1=ri)
                at4 = small.tile([S, NH, S], F32)
                for h in range(NH):
                    nc.sync.dma_start(out=at4[:, h, :],
                                      in_=at[h * S:(h + 1) * S, :])

                u2_ps = ps2.tile([C, C], F32, tag="mm")
                for h in range(NH):
                    nc.tensor.matmul(u2_ps[:, h * S:(h + 1) * S],
                                     lhsT=wvT_sb[:, h, :], rhs=at4[:, h, :],
                                     start=True, stop=True)
                u2_sb = small.tile([C, C], F32)
                nc.scalar.copy(out=u2_sb, in_=u2_ps)
                ut_ps = ps2.tile([C, C], F32, tag="mm")
                nc.tensor.transpose(ut_ps, u2_sb, id32_sb)
                ut_sb = small.tile([C, C], F32)
                nc.scalar.copy(out=ut_sb, in_=ut_ps)
                w2_ps = ps2.tile([C, C], F32, tag="mm")
                nc.tensor.matmul(w2_ps, lhsT=ut_sb, rhs=wf_sb, start=True,
                                 stop=True)
                w2_sb = small.tile([C, C], F16)
                nc.vector.tensor_tensor(out=w2_sb, in0=w2_ps, in1=dg_sb,
                                        op=OP.add)
                ws_ps = ps2.tile([1, C], F32, tag="sm")
                nc.tensor.matmul(ws_ps, lhsT=on16_sb[:, 0:1], rhs=w2_sb,
                                 start=True, stop=True)
                nws_sb = small.tile([1, C], F16)
                nc.vector.tensor_scalar_mul(out=nws_sb, in0=ws_ps, scalar1=-1.0)

            # ====== Phase 3: yp = W2^T xts - w2s (x) rmu; fp16 out ======
            YB = 8            # PSUM banks per block
            nyb = nyc // YB
            with (
                tc.tile_pool(name="psY", bufs=YB, space="PSUM") as psY,
                tc.tile_pool(name="rowq", bufs=2) as rowq,
            ):
                for blk in range(nyb):
                    rmt = rowq.tile([1, YB * YC], F16, name="rmt", tag="rmt")
                    nc.sync.dma_start(
                        out=rmt, in_=rmu16_sb[blk * YB:(blk + 1) * YB, :])
                    yps = []
                    for k in range(YB):
                        q = blk * YB + k
                        tsl = slice(q * YC, (q + 1) * YC)
                        yp = psY.tile([C, YC], F32, tag="y")
                        nc.tensor.matmul(yp, lhsT=w2_sb, rhs=xt_res[:, tsl],
                                         start=True, stop=False)
                        yps.append(yp)
                    for k in range(YB):
                        nc.tensor.matmul(yps[k], lhsT=nws_sb,
                                         rhs=rmt[0:1, k * YC:(k + 1) * YC],
                                         start=False, stop=True)
                    y16 = ybuf.tile([C, YB * YC], F16, name="y16", tag="y16")
                    for k in range(YB):
                        if k % 4 == 3:
                            nc.vector.tensor_copy(
                                out=y16[:, k * YC:(k + 1) * YC], in_=yps[k])
                        else:
                            nc.scalar.copy(out=y16[:, k * YC:(k + 1) * YC],
                                           in_=yps[k])
                    osl = slice(blk * YB * YC, (blk + 1) * YB * YC)
                    nc.sync.dma_start(out=yT_out[:, osl], in_=y16)

    nc.compile()
    return nc


def _numpy_reference(x, gamma, beta, Wq, bq, Wk, bk, Wv, bv, Wf, bf, alpha):
    """Fallback for inputs outside the zero-bias fast path."""
    Bx, Hx, Wx, Cx = x.shape
    t = Hx * Wx
    nh = NH
    s = Cx // nh
    xf = x.reshape(Bx, t, Cx).astype(np.float64)
    mu = xf.mean(-1, keepdims=True)
    var = ((xf - mu) ** 2).mean(-1, keepdims=True)
    xn = (xf - mu) / np.sqrt(var + EPS) * gamma + beta
    Q = (xn @ Wq + bq).reshape(Bx, t, nh, s)
    K = (xn @ Wk + bk).reshape(Bx, t, nh, s)
    V = (xn @ Wv + bv).reshape(Bx, t, nh, s)
    scores = np.einsum("bthi,bthj->bhij", K, Q) / float(alpha)
    scores = scores - scores.max(-1, keepdims=True)
    e = np.exp(scores)
    attn = e / e.sum(-1, keepdims=True)
    out = np.einsum("bthi,bhij->bthj", V, attn).reshape(Bx, t, Cx)
    y = out @ Wf + bf + xn
    return y.reshape(Bx, Hx, Wx, Cx).astype(np.float32)


def make_in_maps(inputs, tloc=TLOC, n_cores=N_CORES):
    x = np.asarray(inputs["x"], dtype=np.float32)
    gamma = np.asarray(inputs["gamma"], dtype=np.float32)
    Wq = np.asarray(inputs["Wq"], dtype=np.float32)
    Wk = np.asarray(inputs["Wk"], dtype=np.float32)
    Wv = np.asarray(inputs["Wv"], dtype=np.float32)
    Wf = np.ascontiguousarray(np.asarray(inputs["Wf"], dtype=np.float32))
    inv_alpha = (1.0 / float(np.asarray(inputs["alpha"]))
                 if "alpha" in inputs else 1.0)

    wq_g = np.ascontiguousarray(gamma[:, None] * Wq * inv_alpha)
    wk_g = np.ascontiguousarray(gamma[:, None] * Wk)
    wv_g = gamma[:, None] * Wv
    wvT4 = np.ascontiguousarray(
        wv_g.T.reshape(NH, S, C).transpose(1, 0, 2).reshape(S, NH * C))
    diag_g = np.ascontiguousarray(np.diag(gamma).astype(np.float32))
    ident_f32 = np.eye(P, dtype=np.float32)
    ident_f16 = np.eye(P, dtype=np.float16)

    w1q = wq_g.sum(axis=0)
    w1q_pk = np.repeat(w1q.reshape(NH, S), S, axis=0).astype(np.float32)
    k1 = wk_g.sum(axis=0)
    k1_col = np.stack([k1, -k1], axis=1).astype(np.float32)
    hsel = (np.arange(C)[None, :] // S == np.arange(NH)[:, None]
            ).astype(np.float32)
    nyc = tloc // YC
    eqsel = np.zeros((P, 2 * nyc - 1), np.float16)
    eqsel[:, nyc - 1] = 1.0     # E_q = eqsel[:, nyc-1-q : 2*nyc-1-q]
    ones16 = np.ones((P, P), np.float16)
    ones32 = np.ones((P, P), np.float32)

    x16 = x.reshape(n_cores, tloc, C).astype(np.float16)
    ngrp = tloc // (P * GRP)
    # x_nat, partition-major, with a zero 129th column per token (the
    # kernel fills it with mu): [cores, P, ngrp * GRP * C1]
    xg = x16.reshape(n_cores, ngrp, GRP, P, C).transpose(0, 3, 1, 2, 4)
    xpad = np.zeros((n_cores, P, ngrp, GRP, C1), np.float16)
    xpad[..., :C] = xg
    x_nat = np.ascontiguousarray(
        xpad.reshape(n_cores, P, ngrp * GRP * C1))
    x_tr = np.ascontiguousarray(x16.transpose(0, 2, 1))

    shared = dict(wq_g=wq_g, wk_g=wk_g, wvT4=wvT4, wf=Wf, diag_gamma=diag_g,
                  ident_f32=ident_f32, ident_f16=ident_f16,
                  w1q_pk=np.ascontiguousarray(w1q_pk),
                  k1_col=np.ascontiguousarray(k1_col),
                  hsel=np.ascontiguousarray(hsel), eqsel=eqsel,
                  ones16=ones16, ones32=ones32)
    return [dict(shared, x_nat=x_nat[i], x_tr=x_tr[i]) for i in range(n_cores)]


_NC_CACHE = {}


def kernel(**inputs) -> np.ndarray:
    zero = lambda k: not np.any(np.asarray(inputs[k]))
    if not (zero("beta") and zero("bq") and zero("bk") and zero("bv")
            and zero("bf")):
        return _numpy_reference(**{k: np.asarray(v) for k, v in inputs.items()})

    key = ("v4", TLOC, N_CORES)
    if key not in _NC_CACHE:
        _NC_CACHE[key] = build_nc(TLOC, N_CORES)
    nc = _NC_CACHE[key]

    in_maps = make_in_maps(inputs)
    res = run_bass_kernel_spmd(nc, in_maps, core_ids=list(range(N_CORES)))
    yT = [res.results[i]["yT16"] for i in range(N_CORES)]
    y = np.concatenate([t.T for t in yT], axis=0).astype(np.float32)
    return np.ascontiguousarray(y.reshape(B, HH, WW, C))


# revision 22
# speedup vs baseline: 1.5702x; 1.0857x over previous
"""MDTA (channel-attention transformer block) Trainium2 kernel, v4.

Math (zero-bias fast path; x16 = fp16(x), per-token mu/r from x16):
  G_needed = sum_t r^2 (x-mu*1)(x-mu*1)^T = G2 - u 1^T - 1 u^T + s 1 1^T
    G2 = sum r^2 x x^T = (r^2 x)^T X,  u = sum r^2 mu x,  s = 1^T u / C
  scores = wk'^T G wq'/alpha (diag 32x32 blocks), attn = softmax
  W2 = diag(g) Wv blockdiag(attn) Wf + diag(gamma),  w2s = 1^T W2
  y_t = r_t(W2^T x16_t) - r_t mu_t w2s = W2^T(x*rb) - w2s (x) rmu_row

Structure (v4 = v3 rescheduled around SBUF residency + overlap):
  Both layouts of x (xT [c, t] and x_nat [t, c]-grouped, host-staged fp16)
  are DMA'd ONCE into SBUF up front (16 MB resident) on the two HWDGE
  rings; a PE warmup burst at t=0 flips HAM to full clock.  Phase A
  (per-token sums via shifted-ones selector matmuls) runs on resident xT
  chunks as they land, in two 32-row halves so stats math / transposes /
  the Gram of half 0 overlap the stats of half 1.  The Gram rhs is the
  resident x_nat row with a 129th column that the kernel fills with mu,
  so u = sum r^2 mu x falls out of the same matmul (no N=1 matmuls).
  zr2 = x*r^2 scaling alternates DVE tensor_scalar / ACT mul (per-
  partition scalars in nat layout).  The pair all-reduce of [G2 | u]
  runs concurrently with all phase-3 prep: xts = xt * r is computed
  in place over the resident xT for all chunks (PE broadcast builds +
  DVE multiplies) while the collective is in flight.  Phase 3 then runs
  8 PSUM banks at a time (batched W2 / rank-1 weight loads), alternating
  ACT/DVE psum->fp16 copies, and writes yT out in 1 MB HWDGE DMAs.

Sharding: 8 cores = (batch 0..3) x (token half 0..1); 66 KB pair all-reduce.
Host does layout/dtype staging only (fp16 casts, the [c, t] transpose,
gamma/alpha folding, final yT.T -> fp32).
"""

import sys

import numpy as np

for _p in ("/opt/trn_rl_repo",):
    if _p not in sys.path:
        sys.path.append(_p)

import concourse.bacc as bacc
import concourse.bass as bass
import concourse.tile as tile
from concourse import mybir
from concourse.bass_utils import run_bass_kernel_spmd

B, HH, WW, C = 4, 256, 256, 128
NH, S = 4, 32
T = HH * WW
N_CORES = 8
TLOC = T // 2
EPS = 1e-5
P = 128
GRP = 4
YC = 512
C1 = C + 1          # x_nat row stride: C channels + mu slot

F32 = mybir.dt.float32
F16 = mybir.dt.float16

AF = mybir.ActivationFunctionType
OP = mybir.AluOpType
AX = mybir.AxisListType


def build_nc(tloc=TLOC, n_cores=N_CORES):
    assert tloc % (P * GRP) == 0 and tloc % YC == 0
    nc = bacc.Bacc("TRN2", target_bir_lowering=False, debug=False,
                   num_devices=n_cores)

    ngrp = tloc // (P * GRP)  # token groups of 512 (= chunks of 512)
    nyc = tloc // YC          # stats row count; == ngrp
    assert nyc == ngrp and nyc <= 64 and nyc % 2 == 0
    HQ = nyc // 2             # rows per stats half

    x_in = nc.declare_dram_parameter("x_nat", [P, ngrp * GRP * C1], F16,
                                     isOutput=False)
    xt_in = nc.declare_dram_parameter("x_tr", [C, tloc], F16, isOutput=False)
    wq_in = nc.declare_dram_parameter("wq_g", [C, C], F32, isOutput=False)
    wk_in = nc.declare_dram_parameter("wk_g", [C, C], F32, isOutput=False)
    wvT_in = nc.declare_dram_parameter("wvT4", [S, NH * C], F32, isOutput=False)
    wf_in = nc.declare_dram_parameter("wf", [C, C], F32, isOutput=False)
    dg_in = nc.declare_dram_parameter("diag_gamma", [C, C], F32, isOutput=False)
    id32_in = nc.declare_dram_parameter("ident_f32", [P, P], F32, isOutput=False)
    id16_in = nc.declare_dram_parameter("ident_f16", [P, P], F16, isOutput=False)
    w1q_in = nc.declare_dram_parameter("w1q_pk", [C, S], F32, isOutput=False)
    k1_in = nc.declare_dram_parameter("k1_col", [C, 2], F32, isOutput=False)
    hsel_in = nc.declare_dram_parameter("hsel", [NH, C], F32, isOutput=False)
    eq_in = nc.declare_dram_parameter("eqsel", [P, 2 * nyc - 1], F16,
                                      isOutput=False)
    on16_in = nc.declare_dram_parameter("ones16", [P, P], F16, isOutput=False)
    on32_in = nc.declare_dram_parameter("ones32", [P, P], F32, isOutput=False)
    yT_out = nc.declare_dram_parameter("yT16", [C, tloc], F16, isOutput=True)

    replica_groups = [[2 * b, 2 * b + 1] for b in range(n_cores // 2)]

    XDMA = 8                 # xT preload transfers (1 MB each)
    NDMA = 16                # x_nat preload transfers (~516 KB each)
    xtw = tloc // XDMA
    ndw = ngrp // NDMA

    with tile.TileContext(nc) as tc:
        with (
            tc.tile_pool(name="const", bufs=1) as const,
            tc.tile_pool(name="sqbuf", bufs=4) as sqbuf,
            tc.tile_pool(name="small", bufs=2) as small,
            tc.tile_pool(name="ybuf", bufs=2) as ybuf,
            tc.tile_pool(name="rows", bufs=2) as rows,
            tc.tile_pool(name="dram", bufs=1, space="DRAM") as dram,
        ):
            # ---- PE warmup burst (HAM -> full clock), no DMA deps ----
            wu_sb = const.tile([P, YC], F16)
            nc.vector.memset(wu_sb, 0.0)
            with tc.tile_pool(name="psW", bufs=1, space="PSUM") as psW:
                wu_ps = psW.tile([P, YC], F32, tag="wu")
                for _ in range(14):
                    nc.tensor.matmul(wu_ps, lhsT=wu_sb[:, 0:P], rhs=wu_sb,
                                     start=True, stop=True)

            # ---- resident x (both layouts) ----
            xt_res = const.tile([C, tloc], F16)
            nat = const.tile([P, ngrp, GRP * C1], F16)
            natf = nat[:].rearrange("p g x -> p (g x)")
            for d in range(XDMA):
                nc.sync.dma_start(
                    out=xt_res[:, d * xtw:(d + 1) * xtw],
                    in_=xt_in[:, d * xtw:(d + 1) * xtw])
            # ---- constants (SWDGE ring; gpsimd idle until collective) ----
            wq_sb = const.tile([C, C], F32)
            wk_sb = const.tile([C, C], F32)
            wvT_sb = const.tile([S, NH, C], F32)
            wf_sb = const.tile([C, C], F32)
            dg_sb = const.tile([C, C], F32)
            id32_sb = const.tile([P, P], F32)
            id16_sb = const.tile([P, P], F16)
            w1q_sb = const.tile([C, S], F32)
            k1_sb = const.tile([C, 2], F32)
            hsel_sb = const.tile([NH, C], F32)
            eq_sb = const.tile([P, 2 * nyc - 1], F16)
            on16_sb = const.tile([P, P], F16)
            on32_sb = const.tile([P, P], F32)
            nc.gpsimd.dma_start(out=id16_sb, in_=id16_in[:])
            nc.gpsimd.dma_start(out=id32_sb, in_=id32_in[:])
            nc.gpsimd.dma_start(out=eq_sb, in_=eq_in[:])
            nc.gpsimd.dma_start(out=wq_sb, in_=wq_in[:])
            nc.gpsimd.dma_start(out=wk_sb, in_=wk_in[:])
            nc.gpsimd.dma_start(out=wvT_sb,
                                in_=wvT_in[:].rearrange("s (h c) -> s h c", h=NH))
            nc.gpsimd.dma_start(out=wf_sb, in_=wf_in[:])
            nc.gpsimd.dma_start(out=dg_sb, in_=dg_in[:])
            nc.gpsimd.dma_start(out=w1q_sb, in_=w1q_in[:])
            nc.gpsimd.dma_start(out=k1_sb, in_=k1_in[:])
            nc.gpsimd.dma_start(out=hsel_sb, in_=hsel_in[:])
            nc.gpsimd.dma_start(out=on16_sb, in_=on16_in[:])
            nc.gpsimd.dma_start(out=on32_sb, in_=on32_in[:])
            eps_sb = const.tile([P, 1], F32)
            nc.vector.memset(eps_sb, EPS)

            nw = ngrp * GRP * C1 // NDMA
            for d in range(NDMA):
                nc.gpsimd.dma_start(out=natf[:, d * nw:(d + 1) * nw],
                                    in_=x_in[:, d * nw:(d + 1) * nw])

            # stats row arrays [nyc, YC] (token t = 512*q + t')
            sx_sb = const.tile([nyc, YC], F32)
            sq_sb = const.tile([nyc, YC], F32)
            scr_sb = const.tile([nyc, YC], F32)
            scr2_sb = const.tile([nyc, YC], F32)
            mu16_sb = const.tile([nyc, YC], F16)
            rmu16_sb = const.tile([nyc, YC], F16)
            r16_sb = const.tile([nyc, YC], F16)
            # column-layout r^2 (fp32, per-partition scalars for zr2)
            r2col = const.tile([P, GRP, ngrp], F32)

            ZRING = 8
            zr2 = const.tile([P, ZRING, C], F16)
            g_sb = small.tile([C, C1], F32)

            with (
                tc.tile_pool(name="psS", bufs=2, space="PSUM") as psS,
                tc.tile_pool(name="ps2", bufs=1, space="PSUM") as ps2,
                tc.tile_pool(name="psG", bufs=1, space="PSUM") as psG,
            ):
                G_ps = psG.tile([C, C1], F32, tag="g")
                nlast = ngrp * GRP - 1
                # ======== Phase A: per-token sums via PE (both halves,
                # emitted back-to-back so PE never idles on stats math) ====
                halves = []
                for mh in range(2):
                    q0 = mh * HQ
                    sx_ps = psS.tile([HQ, YC], F32, tag="sx")
                    sq_ps = psS.tile([HQ, YC], F32, tag="sq")
                    halves.append((q0, sx_ps, sq_ps))
                    for qq in range(HQ):
                        q = q0 + qq
                        xtq = xt_res[:, q * YC:(q + 1) * YC]
                        sqg = sqbuf.tile([C, YC], F16, name="sqg", tag="sq")
                        if q % 2 == 0:
                            nc.vector.tensor_tensor(out=sqg, in0=xtq, in1=xtq,
                                                    op=OP.mult)
                        else:
                            nc.scalar.square(out=sqg, in_=xtq)
                        c0 = nyc - 1 - q + q0
                        eq_v = eq_sb[:, c0:c0 + HQ]
                        nc.tensor.matmul(sx_ps, lhsT=eq_v, rhs=xtq,
                                         start=(qq == 0), stop=(qq == HQ - 1))
                        nc.tensor.matmul(sq_ps, lhsT=eq_v, rhs=sqg,
                                         start=(qq == 0), stop=(qq == HQ - 1))
                for mh in range(2):
                    q0, sx_ps, sq_ps = halves[mh]
                    sl = slice(q0, q0 + HQ)
                    nc.vector.tensor_copy(out=sx_sb[sl], in_=sx_ps)
                    nc.vector.tensor_copy(out=sq_sb[sl], in_=sq_ps)

                    # ---- batched stats math on [HQ, YC] ----
                    nc.vector.tensor_tensor(out=scr_sb[sl], in0=sx_sb[sl],
                                            in1=sx_sb[sl], op=OP.mult)
                    nc.vector.scalar_tensor_tensor(
                        out=scr2_sb[sl], in0=scr_sb[sl],
                        scalar=float(-1.0 / C), in1=sq_sb[sl],
                        op0=OP.mult, op1=OP.add)
                    nc.scalar.activation(out=scr_sb[sl], in_=scr2_sb[sl],
                                         func=AF.Sqrt, bias=eps_sb[0:HQ, :],
                                         scale=float(1.0 / C))
                    nc.vector.reciprocal(out=scr2_sb[sl], in_=scr_sb[sl])
                    nc.scalar.mul(out=mu16_sb[sl], in_=sx_sb[sl],
                                  mul=float(1.0 / C))
                    nc.vector.tensor_tensor(out=rmu16_sb[sl], in0=mu16_sb[sl],
                                            in1=scr2_sb[sl], op=OP.mult)
                    nc.scalar.copy(out=r16_sb[sl], in_=scr2_sb[sl])
                    nc.vector.tensor_tensor(out=scr_sb[sl], in0=scr2_sb[sl],
                                            in1=scr2_sb[sl], op=OP.mult)

                    # column layouts: r^2 -> r2col (fp32), mu -> nat mu slots
                    id_h32 = id32_sb[sl, sl]
                    id_h16 = id16_sb[sl, sl]
                    for j in range(GRP):
                        tpj = ps2.tile([P, HQ], F32, tag="tp")
                        nc.tensor.transpose(tpj, scr_sb[sl, j * P:(j + 1) * P],
                                            id_h32)
                        nc.scalar.copy(out=r2col[:, j, sl], in_=tpj)
                        tpm = ps2.tile([P, HQ], F16, tag="tp")
                        nc.tensor.transpose(tpm, mu16_sb[sl, j * P:(j + 1) * P],
                                            id_h16)
                        nc.scalar.copy(out=nat[:, sl, j * C1 + C], in_=tpm)

                    # ==== Gram for half mh: G2 += (r^2 x)^T [x | mu] ====
                    for g in range(q0, q0 + HQ):
                        for j in range(GRP):
                            i = g * GRP + j
                            r = i % ZRING
                            xnj = nat[:, g, j * C1:j * C1 + C]
                            if i % 4 == 3:
                                nc.scalar.mul(out=zr2[:, r], in_=xnj,
                                              mul=r2col[:, j, g:g + 1])
                            else:
                                nc.vector.tensor_scalar_mul(
                                    out=zr2[:, r], in0=xnj,
                                    scalar1=r2col[:, j, g:g + 1])
                            nc.tensor.matmul(G_ps, lhsT=zr2[:, r],
                                             rhs=nat[:, g, j * C1:(j + 1) * C1],
                                             start=(i == 0), stop=(i == nlast))

                nc.vector.tensor_copy(out=g_sb, in_=G_ps)

            # ============ all-reduce [G2 | u] ============
            g_in_d = dram.tile([C, C1], F32)
            g_out_d = dram.tile([C, C1], F32)
            nc.gpsimd.dma_start(out=g_in_d, in_=g_sb)
            nc.gpsimd.collective_compute(
                "AllReduce", OP.add, replica_groups=replica_groups,
                ins=[g_in_d[:].opt()], outs=[g_out_d[:].opt()])

            # -- overlap: xts = xt * r, in place over resident xT.
            # First half before the softmax block (fills the collective
            # window); second half after it (overlaps early phase 3).
            RB = 8            # chunks per row-remap block

            def xts_block(psR, b):
                rt = rows.tile([1, RB * YC], F16, name="rt", tag="rt")
                nc.sync.dma_start(out=rt,
                                  in_=r16_sb[b * RB:(b + 1) * RB, :])
                for k in range(RB):
                    q = b * RB + k
                    tsl = slice(q * YC, (q + 1) * YC)
                    rb_ps = psR.tile([C, YC], F32, tag="rb")
                    nc.tensor.matmul(rb_ps, lhsT=on16_sb[0:1, :],
                                     rhs=rt[0:1, k * YC:(k + 1) * YC],
                                     start=True, stop=True)
                    rb16 = sqbuf.tile([C, YC], F16, name="rb16", tag="rb")
                    nc.scalar.copy(out=rb16, in_=rb_ps)
                    nc.vector.tensor_tensor(out=xt_res[:, tsl],
                                            in0=xt_res[:, tsl],
                                            in1=rb16, op=OP.mult)

            with tc.tile_pool(name="psR", bufs=4, space="PSUM") as psR:
                for b in range(nyc // RB):
                    xts_block(psR, b)

            gs_sb = small.tile([C, C1], F32)
            nc.gpsimd.dma_start(out=gs_sb, in_=g_out_d)

            # ============ Phase 2: scores + softmax + W2 ============
            with tc.tile_pool(name="ps3", bufs=1, space="PSUM") as ps2:
                u_ap = gs_sb[:, C:C + 1]
                s1_ps = ps2.tile([C, C], F32, tag="mm")
                nc.tensor.matmul(s1_ps, lhsT=gs_sb[:, 0:C], rhs=wq_sb,
                                 start=True, stop=True)   # G symmetric
                s1_sb = small.tile([C, C], F32)
                nc.scalar.copy(out=s1_sb, in_=s1_ps)
                sc_ps = ps2.tile([C, C], F32, tag="mm")
                nc.tensor.matmul(sc_ps, lhsT=wk_sb, rhs=s1_sb, start=True,
                                 stop=True)
                spk = small.tile([P, S], F32)
                for h in range(NH):
                    nc.scalar.copy(out=spk[h * S:(h + 1) * S, :],
                                   in_=sc_ps[h * S:(h + 1) * S,
                                             h * S:(h + 1) * S])

                a_ps = ps2.tile([C, 1], F32, tag="sm")
                nc.tensor.matmul(a_ps, lhsT=wk_sb, rhs=u_ap, start=True,
                                 stop=True)
                a_sb = small.tile([C, 1], F32)
                nc.vector.tensor_copy(out=a_sb, in_=a_ps)
                bc_ps = ps2.tile([C, 1], F32, tag="sm")
                nc.tensor.matmul(bc_ps, lhsT=wq_sb, rhs=u_ap, start=True,
                                 stop=True)
                bc_sb = small.tile([C, 1], F32)
                nc.scalar.copy(out=bc_sb, in_=bc_ps)
                su_ps = ps2.tile([1, 1], F32, tag="sm")
                nc.tensor.matmul(su_ps, lhsT=u_ap, rhs=on32_sb[:, 0:1],
                                 start=True, stop=True)
                su_sb = small.tile([1, 1], F32)
                nc.scalar.copy(out=su_sb, in_=su_ps)
                sc_col_ps = ps2.tile([C, 1], F32, tag="sm")
                nc.tensor.matmul(sc_col_ps, lhsT=on32_sb[0:1, :], rhs=su_sb,
                                 start=True, stop=True)
                scol_sb = small.tile([C, 1], F32)
                nc.scalar.mul(out=scol_sb, in_=sc_col_ps, mul=float(1.0 / C))

                bT_ps = ps2.tile([1, C], F32, tag="sm")
                nc.tensor.transpose(bT_ps, bc_sb, id32_sb)
                bT_sb = small.tile([1, C], F32)
                nc.scalar.copy(out=bT_sb, in_=bT_ps)
                bT4_sb = small.tile([NH, S], F32)
                nc.sync.dma_start(out=bT4_sb, in_=bT_sb)
                bpk_ps = ps2.tile([C, S], F32, tag="sm")
                nc.tensor.matmul(bpk_ps, lhsT=hsel_sb, rhs=bT4_sb,
                                 start=True, stop=True)

                tmp_sb = small.tile([C, 1], F32)
                nc.vector.scalar_tensor_tensor(
                    out=tmp_sb, in0=scol_sb, scalar=k1_sb[:, 0:1], in1=a_sb,
                    op0=OP.mult, op1=OP.subtract)             # s*k1 - a
                s1c = small.tile([P, S], F32)
                nc.vector.scalar_tensor_tensor(
                    out=s1c, in0=w1q_sb, scalar=tmp_sb, in1=spk,
                    op0=OP.mult, op1=OP.add)
                scor = small.tile([P, S], F32)
                nc.vector.scalar_tensor_tensor(
                    out=scor, in0=bpk_ps, scalar=k1_sb[:, 1:2], in1=s1c,
                    op0=OP.mult, op1=OP.add)

                mx = small.tile([P, 1], F32)
                nc.vector.reduce_max(mx, scor, AX.X)
                nmx = small.tile([P, 1], F32)
                nc.vector.tensor_scalar_mul(out=nmx, in0=mx, scalar1=-1.0)
                sh = small.tile([P, S], F32)
                nc.vector.tensor_scalar(out=sh, in0=scor, scalar1=nmx,
                                        scalar2=-87.0, op0=OP.add, op1=OP.max)
                ex = small.tile([P, S], F32)
                es = small.tile([P, 1], F32)
                nc.scalar.activation(out=ex, in_=sh, func=AF.Exp,
                                     bias=0.0, scale=1.0, accum_out=es)
                ri = small.tile([P, 1], F32)
                nc.vector.reciprocal(out=ri, in_=es)
                at = small.tile([P, S], F32)
                nc.vector.tensor_scalar_mul(out=at, in0=ex, scalar1=ri)
                at4 = small.tile([S, NH, S], F32)
                for h in range(NH):
                    nc.sync.dma_start(out=at4[:, h, :],
                                      in_=at[h * S:(h + 1) * S, :])

                u2_ps = ps2.tile([C, C], F32, tag="mm")
                for h in range(NH):
                    nc.tensor.matmul(u2_ps[:, h * S:(h + 1) * S],
                                     lhsT=wvT_sb[:, h, :], rhs=at4[:, h, :],
                                     start=True, stop=True)
                u2_sb = small.tile([C, C], F32)
                nc.scalar.copy(out=u2_sb, in_=u2_ps)
                ut_ps = ps2.tile([C, C], F32, tag="mm")
                nc.tensor.transpose(ut_ps, u2_sb, id32_sb)
                ut_sb = small.tile([C, C], F32)
                nc.scalar.copy(out=ut_sb, in_=ut_ps)
                w2_ps = ps2.tile([C, C], F32, tag="mm")
                nc.tensor.matmul(w2_ps, lhsT=ut_sb, rhs=wf_sb, start=True,
                                 stop=True)
                w2_sb = small.tile([C, C], F16)
                nc.vector.tensor_tensor(out=w2_sb, in0=w2_ps, in1=dg_sb,
                                        op=OP.add)
                ws_ps = ps2.tile([1, C], F32, tag="sm")
                nc.tensor.matmul(ws_ps, lhsT=on16_sb[:, 0:1], rhs=w2_sb,
                                 start=True, stop=True)
                nws_sb = small.tile([1, C], F16)
                nc.vector.tensor_scalar_mul(out=nws_sb, in0=ws_ps, scalar1=-1.0)

            # ====== Phase 3: yp = W2^T xts - w2s (x) rmu; fp16 out ======
            YB = 8            # PSUM banks per block
            nyb = nyc // YB
            with tc.tile_pool(name="psY", bufs=YB, space="PSUM") as psY:
                for blk in range(nyb):
                    rmt = rows.tile([1, YB * YC], F16, name="rmt", tag="rt")
                    nc.sync.dma_start(
                        out=rmt, in_=rmu16_sb[blk * YB:(blk + 1) * YB, :])
                    yps = []
                    for k in range(YB):
                        q = blk * YB + k
                        tsl = slice(q * YC, (q + 1) * YC)
                        yp = psY.tile([C, YC], F32, tag="y")
                        nc.tensor.matmul(yp, lhsT=w2_sb, rhs=xt_res[:, tsl],
                                         start=True, stop=False)
                        yps.append(yp)
                    for k in range(YB):
                        nc.tensor.matmul(yps[k], lhsT=nws_sb,
                                         rhs=rmt[0:1, k * YC:(k + 1) * YC],
                                         start=False, stop=True)
                    y16 = ybuf.tile([C, YB * YC], F16, name="y16", tag="y16")
                    for k in range(YB):
                        if k % 4 == 3:
                            nc.vector.tensor_copy(
                                out=y16[:, k * YC:(k + 1) * YC], in_=yps[k])
                        else:
                            nc.scalar.copy(out=y16[:, k * YC:(k + 1) * YC],
                                           in_=yps[k])
                    osl = slice(blk * YB * YC, (blk + 1) * YB * YC)
                    nc.sync.dma_start(out=yT_out[:, osl], in_=y16)

    nc.compile()
    return nc


def _numpy_reference(x, gamma, beta, Wq, bq, Wk, bk, Wv, bv, Wf, bf, alpha):
    """Fallback for inputs outside the zero-bias fast path."""
    Bx, Hx, Wx, Cx = x.shape
    t = Hx * Wx
    nh = NH
    s = Cx // nh
    xf = x.reshape(Bx, t, Cx).astype(np.float64)
    mu = xf.mean(-1, keepdims=True)
    var = ((xf - mu) ** 2).mean(-1, keepdims=True)
    xn = (xf - mu) / np.sqrt(var + EPS) * gamma + beta
    Q = (xn @ Wq + bq).reshape(Bx, t, nh, s)
    K = (xn @ Wk + bk).reshape(Bx, t, nh, s)
    V = (xn @ Wv + bv).reshape(Bx, t, nh, s)
    scores = np.einsum("bthi,bthj->bhij", K, Q) / float(alpha)
    scores = scores - scores.max(-1, keepdims=True)
    e = np.exp(scores)
    attn = e / e.sum(-1, keepdims=True)
    out = np.einsum("bthi,bhij->bthj", V, attn).reshape(Bx, t, Cx)
    y = out @ Wf + bf + xn
    return y.reshape(Bx, Hx, Wx, Cx).astype(np.float32)


def make_in_maps(inputs, tloc=TLOC, n_cores=N_CORES):
    x = np.asarray(inputs["x"], dtype=np.float32)
    gamma = np.asarray(inputs["gamma"], dtype=np.float32)
    Wq = np.asarray(inputs["Wq"], dtype=np.float32)
    Wk = np.asarray(inputs["Wk"], dtype=np.float32)
    Wv = np.asarray(inputs["Wv"], dtype=np.float32)
    Wf = np.ascontiguousarray(np.asarray(inputs["Wf"], dtype=np.float32))
    inv_alpha = (1.0 / float(np.asarray(inputs["alpha"]))
                 if "alpha" in inputs else 1.0)

    wq_g = np.ascontiguousarray(gamma[:, None] * Wq * inv_alpha)
    wk_g = np.ascontiguousarray(gamma[:, None] * Wk)
    wv_g = gamma[:, None] * Wv
    wvT4 = np.ascontiguousarray(
        wv_g.T.reshape(NH, S, C).transpose(1, 0, 2).reshape(S, NH * C))
    diag_g = np.ascontiguousarray(np.diag(gamma).astype(np.float32))
    ident_f32 = np.eye(P, dtype=np.float32)
    ident_f16 = np.eye(P, dtype=np.float16)

    w1q = wq_g.sum(axis=0)
    w1q_pk = np.repeat(w1q.reshape(NH, S), S, axis=0).astype(np.float32)
    k1 = wk_g.sum(axis=0)
    k1_col = np.stack([k1, -k1], axis=1).astype(np.float32)
    hsel = (np.arange(C)[None, :] // S == np.arange(NH)[:, None]
            ).astype(np.float32)
    nyc = tloc // YC
    eqsel = np.zeros((P, 2 * nyc - 1), np.float16)
    eqsel[:, nyc - 1] = 1.0     # E_q = eqsel[:, nyc-1-q : 2*nyc-1-q]
    ones16 = np.ones((P, P), np.float16)
    ones32 = np.ones((P, P), np.float32)

    x16 = x.reshape(n_cores, tloc, C).astype(np.float16)
    ngrp = tloc // (P * GRP)
    # x_nat, partition-major, with a zero 129th column per token (the
    # kernel fills it with mu): [cores, P, ngrp * GRP * C1]
    xg = x16.reshape(n_cores, ngrp, GRP, P, C).transpose(0, 3, 1, 2, 4)
    xpad = np.zeros((n_cores, P, ngrp, GRP, C1), np.float16)
    xpad[..., :C] = xg
    x_nat = np.ascontiguousarray(
        xpad.reshape(n_cores, P, ngrp * GRP * C1))
    x_tr = np.ascontiguousarray(x16.transpose(0, 2, 1))

    shared = dict(wq_g=wq_g, wk_g=wk_g, wvT4=wvT4, wf=Wf, diag_gamma=diag_g,
                  ident_f32=ident_f32, ident_f16=ident_f16,
                  w1q_pk=np.ascontiguousarray(w1q_pk),
                  k1_col=np.ascontiguousarray(k1_col),
                  hsel=np.ascontiguousarray(hsel), eqsel=eqsel,
                  ones16=ones16, ones32=ones32)
    return [dict(shared, x_nat=x_nat[i], x_tr=x_tr[i]) for i in range(n_cores)]


_NC_CACHE = {}


def kernel(**inputs) -> np.ndarray:
    zero = lambda k: not np.any(np.asarray(inputs[k]))
    if not (zero("beta") and zero("bq") and zero("bk") and zero("bv")
            and zero("bf")):
        return _numpy_reference(**{k: np.asarray(v) for k, v in inputs.items()})

    key = ("v4", TLOC, N_CORES)
    if key not in _NC_CACHE:
        _NC_CACHE[key] = build_nc(TLOC, N_CORES)
    nc = _NC_CACHE[key]

    in_maps = make_in_maps(inputs)
    res = run_bass_kernel_spmd(nc, in_maps, core_ids=list(range(N_CORES)))
    yT = [res.results[i]["yT16"] for i in range(N_CORES)]
    y = np.concatenate([t.T for t in yT], axis=0).astype(np.float32)
    return np.ascontiguousarray(y.reshape(B, HH, WW, C))
